# revision 5
# baseline (speedup 1.0000x reference)
"""Trainium2 Bass kernel for nn_Attention_70136815943694.

Attention with the reference's source bug preserved (K uses the V
projection). x:[2,2048,1024], 16 heads x 64 dim. Sharded over 8
NeuronCores as (batch x head-group): core c handles batch c//4 and
heads [4*(c%4) .. 4*(c%4)+3]. Each core's output slice is independent,
so there are no collectives; the host shards inputs and reassembles.

Per-core device computation (d-major layouts to avoid transposes):
  QT = wqT.T @ xT (+bq)           [256, 2048]   (bf16 in, fp32 psum)
  KVT = wvT.T @ xT (+bv)          [256, 2048]
  Vn  = xT.T @ wvT (+bv row)      [2048, 256]   natural layout for PV
  per head h, s1 half:
    PT_j  = exp(0.125 * KT_j.T @ QT_h)          [128, 1024] per s2 chunk
    attnT += [V_h | 1].T @ PT_j                 [65, 1024]  (row 64 = denom)
    out   = attnT[:64] / denom                  (gpsimd bcast + DVE)
"""
import os

import numpy as np
import ml_dtypes

B = 2
S = 2048
D = 1024
NH = 16
HD = 64
N_CORES = 8
HEADS_PER_CORE = 4
DPC = HEADS_PER_CORE * HD  # 256 projection rows per core
P = 128
KC = D // P  # 8 contraction chunks
SC = S // P  # 16 sequence chunks
S1H = S // 2  # 1024, s1 half width

_NC_CACHE = {}


def build_nc():
    if "nc" in _NC_CACHE:
        return _NC_CACHE["nc"]
    import concourse.bass as bass
    import concourse.mybir as mybir
    import concourse.tile as tile
    from concourse import bacc

    BF16 = mybir.dt.bfloat16
    F32 = mybir.dt.float32
    Act = mybir.ActivationFunctionType
    ts = bass.ts

    nc = bacc.Bacc(None, target_bir_lowering=False, debug=False)
    xT_d = nc.declare_dram_parameter("xT", [D, S], BF16, isOutput=False)
    wqT_d = nc.declare_dram_parameter("wqT", [D, DPC], BF16, isOutput=False)
    wvT_d = nc.declare_dram_parameter("wvT", [D, DPC], BF16, isOutput=False)
    bq_d = nc.declare_dram_parameter("bq", [DPC, 1], F32, isOutput=False)
    bv_row_d = nc.declare_dram_parameter("bv_row", [1, DPC], BF16, isOutput=False)
    out_d = nc.declare_dram_parameter("out", [DPC, S], F32, isOutput=True)

    with tile.TileContext(nc) as tc:
        with (
            tc.tile_pool(name="persist", bufs=1) as persist,
            tc.tile_pool(name="pt", bufs=3) as pt_pool,
            tc.tile_pool(name="epi", bufs=2) as epi_pool,
        ):
            # ---- load inputs -------------------------------------------------
            wq_sb = [persist.tile([P, DPC], BF16, name=f"wq{k}", tag=f"wq{k}") for k in range(KC)]
            wv_sb = [persist.tile([P, DPC], BF16, name=f"wv{k}", tag=f"wv{k}") for k in range(KC)]
            for k in range(KC):
                nc.sync.dma_start(wq_sb[k][:], wqT_d[ts(k, P), :])
                nc.sync.dma_start(wv_sb[k][:], wvT_d[ts(k, P), :])
            bq_sb = [persist.tile([P, 1], F32, name=f"bq{m}", tag=f"bq{m}") for m in range(2)]
            for m in range(2):
                nc.sync.dma_start(bq_sb[m][:], bq_d[ts(m, P), :])
            bv_row_sb = persist.tile([1, DPC], BF16, tag="bvrow")
            nc.sync.dma_start(bv_row_sb[:], bv_row_d[:])
            ones_row = persist.tile([1, P], BF16, tag="ones")
            nc.vector.memset(ones_row[:], 1.0)
            xt_sb = [persist.tile([P, S], BF16, name=f"xt{k}", tag=f"xt{k}") for k in range(KC)]
            for k in range(KC):
                nc.sync.dma_start(xt_sb[k][:], xT_d[ts(k, P), :])

            # ---- projections QT, KVT (d-major) ------------------------------
            qT_sb = [persist.tile([P, S], BF16, name=f"qT{m}", tag=f"qT{m}") for m in range(2)]
            kvT_sb = [persist.tile([P, S], BF16, name=f"kvT{m}", tag=f"kvT{m}") for m in range(2)]
            bv_col_sb = [persist.tile([P, 1], F32, name=f"bvc{m}", tag=f"bvc{m}") for m in range(2)]
            bv_d = nc.declare_dram_parameter("bv", [DPC, 1], F32, isOutput=False)
            for m in range(2):
                nc.sync.dma_start(bv_col_sb[m][:], bv_d[ts(m, P), :])
            with tc.tile_pool(name="psum_proj", bufs=2, space="PSUM") as psum_proj:
                for w_sb, dst_sb, bias_sb in (
                    (wq_sb, qT_sb, bq_sb),
                    (wv_sb, kvT_sb, bv_col_sb),
                ):
                    for m in range(2):
                        ps = psum_proj.tile([P, S], F32, tag="proj")
                        for k in range(KC):
                            for n in range(S // 512):
                                nc.tensor.matmul(
                                    ps[:, ts(n, 512)],
                                    w_sb[k][:, ts(m, P)],
                                    xt_sb[k][:, ts(n, 512)],
                                    start=(k == 0),
                                    stop=(k == KC - 1),
                                )
                        nc.scalar.activation(
                            dst_sb[m][:], ps[:], Act.Identity, bias=bias_sb[m][:], scale=1.0
                        )

            # ---- V natural layout (+ ones column), with bias ----------------
            # v_sb[j][:, h, 0:64] = Vn chunk for head h; [., ., 64] = 1.0
            v_sb = [
                persist.tile([P, HEADS_PER_CORE, HD + 1], BF16, name=f"v{j}", tag=f"v{j}")
                for j in range(SC)
            ]
            with tc.tile_pool(name="psum_v", bufs=4, space="PSUM") as psum_v:
                for j in range(SC):
                    ps = psum_v.tile([P, DPC], F32, tag="vnat")
                    for k in range(KC):
                        nc.tensor.matmul(
                            ps[:],
                            xt_sb[k][:, ts(j, P)],
                            wv_sb[k][:],
                            start=(k == 0),
                            stop=False,
                        )
                    # += bv (broadcast along s) via K=1 ones-row matmul
                    nc.tensor.matmul(
                        ps[:],
                        ones_row[:],
                        bv_row_sb[:],
                        start=False,
                        stop=True,
                    )
                    nc.vector.memset(v_sb[j][:, :, HD : HD + 1], 1.0)
                    nc.vector.tensor_copy(
                        v_sb[j][:, :, 0:HD],
                        ps[:].rearrange("p (h d) -> p h d", h=HEADS_PER_CORE),
                    )

            # ---- attention ---------------------------------------------------
            with (
                tc.tile_pool(name="psum_sc", bufs=2, space="PSUM") as psum_sc,
                tc.tile_pool(name="psum_at", bufs=2, space="PSUM") as psum_at,
            ):
                for h in range(HEADS_PER_CORE):
                    mi, hh = divmod(h, 2)
                    r0 = hh * HD
                    for s1h in range(2):
                        at = psum_at.tile([HD + 1, S1H], F32, tag="at")
                        for j in range(SC):
                            sc = psum_sc.tile([P, S1H], F32, tag="sc")
                            for n in range(S1H // 512):
                                nc.tensor.matmul(
                                    sc[:, ts(n, 512)],
                                    kvT_sb[mi][r0 : r0 + HD, ts(j, P)],
                                    qT_sb[mi][
                                        r0 : r0 + HD,
                                        s1h * S1H + n * 512 : s1h * S1H + (n + 1) * 512,
                                    ],
                                    start=True,
                                    stop=True,
                                )
                            pt = pt_pool.tile([P, S1H], BF16, tag="pt")
                            nc.scalar.activation(pt[:], sc[:], Act.Exp, scale=0.125)
                            for n in range(S1H // 512):
                                nc.tensor.matmul(
                                    at[:, ts(n, 512)],
                                    v_sb[j][:, h, :],
                                    pt[:, ts(n, 512)],
                                    start=(j == 0),
                                    stop=(j == SC - 1),
                                )
                        # epilogue: divide by denom (row HD of at)
                        denom = epi_pool.tile([1, S1H], F32, tag="denom")
                        nc.vector.tensor_copy(denom[:], at[HD : HD + 1, :])
                        bcast = epi_pool.tile([HD, S1H], F32, tag="bcast")
                        nc.gpsimd.partition_broadcast(bcast[:], denom[:])
                        recip = epi_pool.tile([HD, S1H], F32, tag="recip")
                        nc.vector.reciprocal(recip[:], bcast[:])
                        outt = epi_pool.tile([HD, S1H], F32, tag="outt")
                        nc.vector.tensor_mul(outt[:], at[0:HD, :], recip[:])
                        nc.sync.dma_start(
                            out_d[ts(h, HD), ts(s1h, S1H)],
                            outt[:],
                        )

    nc.compile()
    _NC_CACHE["nc"] = nc
    return nc


def shard_inputs(x, Wq, bq, Wv, bv):
    bf16 = ml_dtypes.bfloat16
    x = np.asarray(x, dtype=np.float32)
    Wq = np.asarray(Wq, dtype=np.float32)
    bq = np.asarray(bq, dtype=np.float32)
    Wv = np.asarray(Wv, dtype=np.float32)
    bv = np.asarray(bv, dtype=np.float32)
    in_maps = []
    xT = [np.ascontiguousarray(x[b].T).astype(bf16) for b in range(B)]
    for c in range(N_CORES):
        b, g = divmod(c, N_CORES // B)
        heads = [HEADS_PER_CORE * g + hl for hl in range(HEADS_PER_CORE)]
        perm = np.array([i * NH + h for h in heads for i in range(HD)])
        in_maps.append(
            {
                "xT": xT[b],
                "wqT": np.ascontiguousarray(Wq[perm, :].T).astype(bf16),
                "wvT": np.ascontiguousarray(Wv[perm, :].T).astype(bf16),
                "bq": np.ascontiguousarray(bq[perm].reshape(DPC, 1)),
                "bv": np.ascontiguousarray(bv[perm].reshape(DPC, 1)),
                "bv_row": np.ascontiguousarray(bv[perm].reshape(1, DPC)).astype(bf16),
            }
        )
    return in_maps


def assemble(results):
    out = np.empty((B, S, D), dtype=np.float32)
    for c in range(N_CORES):
        b, g = divmod(c, N_CORES // B)
        out[b][:, g * DPC : (g + 1) * DPC] = results[c]["out"].T
    return out


def kernel(x, Wq, bq, Wv, bv):
    from concourse.bass_utils import run_bass_kernel_spmd

    nc = build_nc()
    in_maps = shard_inputs(x, Wq, bq, Wv, bv)
    res = run_bass_kernel_spmd(nc, in_maps, core_ids=list(range(N_CORES)))
    return assemble(res.results)


if __name__ == "__main__":
    rng = np.random.default_rng(0)
    inputs = {
        "x": rng.standard_normal((B, S, D), dtype=np.float32),
        "Wq": (rng.standard_normal((D, D), dtype=np.float32) / 32.0),
        "bq": rng.standard_normal(D, dtype=np.float32) * 0.02,
        "Wv": (rng.standard_normal((D, D), dtype=np.float32) / 32.0),
        "bv": rng.standard_normal(D, dtype=np.float32) * 0.02,
    }
    out = kernel(**inputs)
    print("kernel ran, out shape:", out.shape)


# revision 9
# speedup vs baseline: 1.3597x; 1.3597x over previous
"""Trainium2 Bass kernel for nn_Attention_70136815943694.

Attention with the reference's source bug preserved (K uses the V
projection). x:[2,2048,1024], 16 heads x 64 dim. Sharded over 8
NeuronCores as (batch x head-group): core c handles batch c//4 and
heads [4*(c%4) .. 4*(c%4)+3]. Each core's output slice is independent,
so there are no collectives; the host shards inputs and reassembles.

Per-core device pipeline (d-major layouts, no on-device transposes of
big tensors; V is produced by DMA-transposing the KV projection):
  QT = wqT.T @ xT (+bq)     [256, 2048] bf16   (DVE evac w/ bias)
  KVT = wvT.T @ xT (+bv)    [256, 2048] bf16
  V   = dma_transpose(KVT)  [2048, 4, 65] bf16 (ones col appended)
  per head-pair p, s1 quarter q (512 wide), s2 chunk j (128):
    scores: two K=64 matmuls row-packed via tile_position (0,0)/(64,0)
    PT = exp(0.125*scores) on ACT, one [128,1024] op for both heads
    atH += [V_h | 1].T @ PT_h   [65, 512] psum, row 64 = softmax denom
  epilogue: gpsimd partition_broadcast denom, DVE fast reciprocal, mul
The m=1 half of the projections and the pair-1 V transposes are
interleaved into pair-0's ACT-bound attention phase.
"""
import numpy as np
import ml_dtypes

B = 2
S = 2048
D = 1024
NH = 16
HD = 64
N_CORES = 8
HEADS_PER_CORE = 4
DPC = HEADS_PER_CORE * HD  # 256 projection rows per core
P = 128
KC = D // P  # 8 contraction chunks
SC = S // P  # 16 s2 chunks
SQ = 512  # s1 quarter width
USE_TILE_POS = True
NSQ = S // SQ  # 4

_NC_CACHE = {}


def build_nc():
    if "nc" in _NC_CACHE:
        return _NC_CACHE["nc"]
    import concourse.bass as bass
    import concourse.mybir as mybir
    import concourse.tile as tile
    from concourse import bacc

    BF16 = mybir.dt.bfloat16
    F32 = mybir.dt.float32
    Act = mybir.ActivationFunctionType
    ts = bass.ts

    nc = bacc.Bacc(None, target_bir_lowering=False, debug=False)
    xT_d = nc.declare_dram_parameter("xT", [D, S], BF16, isOutput=False)
    wqT_d = nc.declare_dram_parameter("wqT", [D, DPC], BF16, isOutput=False)
    wvT_d = nc.declare_dram_parameter("wvT", [D, DPC], BF16, isOutput=False)
    bq_d = nc.declare_dram_parameter("bq", [DPC, 1], F32, isOutput=False)
    bv_d = nc.declare_dram_parameter("bv", [DPC, 1], F32, isOutput=False)
    out_d = nc.declare_dram_parameter("out", [DPC, S], F32, isOutput=True)

    with tile.TileContext(nc) as tc:
        with (
            tc.tile_pool(name="persist", bufs=1) as persist,
            tc.tile_pool(name="pt", bufs=3) as pt_pool,
            tc.tile_pool(name="epi", bufs=2) as epi_pool,
        ):
            # warm the ACT exp table set at t~0 so the one-time table load
            # overlaps the input DMAs
            warm = persist.tile([1, 8], F32, tag="warm")
            nc.vector.memset(warm[:], 0.0)
            nc.scalar.activation(warm[:], warm[:], Act.Exp, scale=1.0)

            # ---- input loads -------------------------------------------------
            xt_sb = [
                persist.tile([P, S], BF16, name=f"xt{k}", tag=f"xt{k}")
                for k in range(KC)
            ]
            for k in range(KC):
                nc.sync.dma_start(xt_sb[k][:], xT_d[ts(k, P), :])
            wq_sb = [
                persist.tile([P, DPC], BF16, name=f"wq{k}", tag=f"wq{k}")
                for k in range(KC)
            ]
            wv_sb = [
                persist.tile([P, DPC], BF16, name=f"wv{k}", tag=f"wv{k}")
                for k in range(KC)
            ]
            for k in range(KC):
                nc.sync.dma_start(wq_sb[k][:], wqT_d[ts(k, P), :])
                nc.sync.dma_start(wv_sb[k][:], wvT_d[ts(k, P), :])
            bq_sb = [
                persist.tile([P, 1], F32, name=f"bq{m}", tag=f"bq{m}")
                for m in range(2)
            ]
            bv_sb = [
                persist.tile([P, 1], F32, name=f"bv{m}", tag=f"bv{m}")
                for m in range(2)
            ]
            for m in range(2):
                nc.sync.dma_start(bq_sb[m][:], bq_d[ts(m, P), :])
                nc.sync.dma_start(bv_sb[m][:], bv_d[ts(m, P), :])

            qT_sb = [
                persist.tile([P, S], BF16, name=f"qT{m}", tag=f"qT{m}")
                for m in range(2)
            ]
            kvT_sb = [
                persist.tile([P, S], BF16, name=f"kvT{m}", tag=f"kvT{m}")
                for m in range(2)
            ]
            # v_sb[p][hl][j]: [128, 65] = V chunk j for head 2p+hl,
            # column 64 = 1.0 (softmax denominator trick). One tile per head
            # so the transpose-DMA destination starts at a free offset of 0
            # (offset-65 bf16 destinations silently corrupt the transpose).
            v_sb = [
                [
                    [
                        persist.tile(
                            [P, HD + 1], BF16,
                            name=f"v{p}_{hl}_{j}", tag=f"v{p}_{hl}_{j}",
                        )
                        for j in range(SC)
                    ]
                    for hl in range(2)
                ]
                for p in range(2)
            ]

            def emit_proj(w_sb, dst, bias, m, psum_pool, half=None):
                """One projection m-chunk (or one 1024-wide half of it)."""
                halves = (0, 1) if half is None else (half,)
                for hf in halves:
                    ps = psum_pool.tile([P, 1024], F32, tag="proj", name="pp")
                    for k in range(KC):
                        for n in range(2):
                            nc.tensor.matmul(
                                ps[:, ts(n, 512)],
                                w_sb[k][:, ts(m, P)],
                                xt_sb[k][:, ts(hf * 2 + n, 512)],
                                start=(k == 0),
                                stop=(k == KC - 1),
                            )
                    nc.vector.tensor_scalar_add(
                        dst[:, ts(hf, 1024)], ps[:], bias[:]
                    )

            def emit_vtrans(p, j):
                """Build v_sb[p][.][j] by DMA-transposing two KVT head slices."""
                for hl in range(2):
                    nc.sync.dma_start_transpose(
                        v_sb[p][hl][j][:, 0:HD],
                        kvT_sb[p][hl * HD : (hl + 1) * HD, ts(j, P)],
                    )
                    nc.vector.memset(v_sb[p][hl][j][:, HD : HD + 1], 1.0)

            def proj_steps(w_sb, dst, bias, m, psum_pool):
                """Generator: emit projection m-chunk in small filler steps."""
                for hf in range(2):
                    ps = psum_pool.tile([P, 1024], F32, tag="proj", name="pp")
                    for k in range(KC):
                        for n in range(2):
                            nc.tensor.matmul(
                                ps[:, ts(n, 512)],
                                w_sb[k][:, ts(m, P)],
                                xt_sb[k][:, ts(hf * 2 + n, 512)],
                                start=(k == 0),
                                stop=(k == KC - 1),
                            )
                        yield
                    nc.vector.tensor_scalar_add(
                        dst[:, ts(hf, 1024)], ps[:], bias[:]
                    )
                    yield

            def vtrans_steps(p):
                for j in range(SC):
                    emit_vtrans(p, j)
                    if j % 4 == 3:
                        yield

            # ---- prologue: m=0 projections (heads 0,1) ----------------------
            with tc.tile_pool(name="psum_pro", bufs=2, space="PSUM") as psum_pro:
                emit_proj(wq_sb, qT_sb[0], bq_sb[0], 0, psum_pro)
                emit_proj(wv_sb, kvT_sb[0], bv_sb[0], 0, psum_pro)

            # ---- attention ---------------------------------------------------
            with (
                tc.tile_pool(name="psum_sc", bufs=2, space="PSUM") as psum_sc,
                tc.tile_pool(name="psum_at", bufs=2, space="PSUM") as psum_at,
                tc.tile_pool(name="psum_mi", bufs=1, space="PSUM") as psum_mi,
            ):
                for j in range(SC):
                    emit_vtrans(0, j)

                fillers = {
                    (0, 1): [proj_steps(wv_sb, kvT_sb[1], bv_sb[1], 1, psum_mi)],
                    (0, 2): [vtrans_steps(1),
                             proj_steps(wq_sb, qT_sb[1], bq_sb[1], 1, psum_mi)],
                    (0, 3): [],
                }

                for p in range(2):
                    for q in range(NSQ):
                        gens = fillers.get((p, q), [])
                        at = [
                            psum_at.tile([HD + 1, SQ], F32, tag="at", name="at")
                            for _ in range(2)
                        ]
                        for j in range(SC):
                            sc = psum_sc.tile([P, 1024], F32, tag="sc", name="sc")
                            for hl in range(2):
                                nc.tensor.matmul(
                                    sc[:, ts(hl, SQ)],
                                    kvT_sb[p][hl * HD : (hl + 1) * HD, ts(j, P)],
                                    qT_sb[p][
                                        hl * HD : (hl + 1) * HD, ts(q, SQ)
                                    ],
                                    start=True,
                                    stop=True,
                                    tile_position=(hl * HD, 0) if USE_TILE_POS else None,
                                )
                            pt = pt_pool.tile([P, 1024], BF16, tag="pt", name="pt")
                            nc.scalar.activation(pt[:], sc[:], Act.Exp, scale=0.125)
                            for hl in range(2):
                                nc.tensor.matmul(
                                    at[hl][:],
                                    v_sb[p][hl][j][:],
                                    pt[:, ts(hl, SQ)],
                                    start=(j == 0),
                                    stop=(j == SC - 1),
                                )
                            # filler work (m=1 projections, pair-1 V builds)
                            steps = 0
                            while gens and steps < 2:
                                try:
                                    next(gens[0])
                                    steps += 1
                                except StopIteration:
                                    gens.pop(0)
                        # epilogue per head
                        for hl in range(2):
                            head = 2 * p + hl
                            asb = epi_pool.tile([HD, SQ], F32, tag="asb", name="asb")
                            nc.vector.tensor_copy(asb[:], at[hl][0:HD, :])
                            # partition_broadcast reads the tensor's partition 0
                            # regardless of AP offset: stage the denom row in a
                            # dedicated p0 tile first.
                            dr = epi_pool.tile([1, SQ], F32, tag="dr", name="dr")
                            nc.vector.tensor_copy(dr[:], at[hl][HD : HD + 1, :])
                            bc = epi_pool.tile([HD, SQ], F32, tag="bc", name="bc")
                            nc.gpsimd.partition_broadcast(bc[:], dr[:])
                            rc = epi_pool.tile([HD, SQ], F32, tag="rc", name="rc")
                            nc.vector.reciprocal_approx_fast(rc[:], bc[:])
                            ot = epi_pool.tile([HD, SQ], F32, tag="ot", name="ot")
                            nc.vector.tensor_mul(ot[:], asb[:], rc[:])
                            nc.gpsimd.dma_start(
                                out_d[ts(head, HD), ts(q, SQ)], ot[:]
                            )

    nc.compile()
    _NC_CACHE["nc"] = nc
    return nc


def shard_inputs(x, Wq, bq, Wv, bv):
    bf16 = ml_dtypes.bfloat16
    x = np.asarray(x, dtype=np.float32)
    Wq = np.asarray(Wq, dtype=np.float32)
    bq = np.asarray(bq, dtype=np.float32)
    Wv = np.asarray(Wv, dtype=np.float32)
    bv = np.asarray(bv, dtype=np.float32)
    in_maps = []
    xT = [np.ascontiguousarray(x[b].T).astype(bf16) for b in range(B)]
    for c in range(N_CORES):
        b, g = divmod(c, N_CORES // B)
        heads = [HEADS_PER_CORE * g + hl for hl in range(HEADS_PER_CORE)]
        perm = np.array([i * NH + h for h in heads for i in range(HD)])
        in_maps.append(
            {
                "xT": xT[b],
                "wqT": np.ascontiguousarray(Wq[perm, :].T).astype(bf16),
                "wvT": np.ascontiguousarray(Wv[perm, :].T).astype(bf16),
                "bq": np.ascontiguousarray(bq[perm].reshape(DPC, 1)),
                "bv": np.ascontiguousarray(bv[perm].reshape(DPC, 1)),
            }
        )
    return in_maps


def assemble(results):
    out = np.empty((B, S, D), dtype=np.float32)
    for c in range(N_CORES):
        b, g = divmod(c, N_CORES // B)
        out[b][:, g * DPC : (g + 1) * DPC] = results[c]["out"].T
    return out


def kernel(x, Wq, bq, Wv, bv):
    from concourse.bass_utils import run_bass_kernel_spmd

    nc = build_nc()
    in_maps = shard_inputs(x, Wq, bq, Wv, bv)
    res = run_bass_kernel_spmd(nc, in_maps, core_ids=list(range(N_CORES)))
    return assemble(res.results)


if __name__ == "__main__":
    rng = np.random.default_rng(0)
    inputs = {
        "x": rng.standard_normal((B, S, D), dtype=np.float32),
        "Wq": (rng.standard_normal((D, D), dtype=np.float32) / 32.0),
        "bq": rng.standard_normal(D, dtype=np.float32) * 0.02,
        "Wv": (rng.standard_normal((D, D), dtype=np.float32) / 32.0),
        "bv": rng.standard_normal(D, dtype=np.float32) * 0.02,
    }
    out = kernel(**inputs)
    print("kernel ran, out shape:", out.shape)


# revision 10
# speedup vs baseline: 1.4438x; 1.0618x over previous
"""Trainium2 Bass kernel for nn_Attention_70136815943694.

Attention with the reference's source bug preserved (K uses the V
projection). x:[2,2048,1024], 16 heads x 64 dim. Sharded over 8
NeuronCores as (batch x head-group): core c handles batch c//4 and
heads [4*(c%4) .. 4*(c%4)+3]. Each core's output slice is independent,
so there are no collectives; the host shards inputs and reassembles.

Per-core device pipeline (d-major layouts, no transposes):
  QT = wqT.T @ xT (+bq)     [256, 2048] bf16   (DVE evac w/ bias)
  KVT = wvT.T @ xT (+bv)    [256, 2048] bf16
  Vn  = xT.T @ wvT (+bv via K=1 ones matmul)  [2048, 4x65] bf16
  per head-pair p, s1 quarter q (512 wide), s2 chunk j (128):
    scores: two K=64 matmuls row-packed via tile_position (0,0)/(64,0)
    PT = exp(0.125*scores) on ACT, one [128,1024] op for both heads
    atH += [V_h | 1].T @ PT_h   [65, 512] psum, row 64 = softmax denom
  epilogue: gpsimd partition_broadcast denom, DVE fast reciprocal, mul
Vn chunks are interleaved into pair-0's first ACT-bound group; the m=1
projections fill PE slack in later pair-0 groups.
"""
import numpy as np
import ml_dtypes

B = 2
S = 2048
D = 1024
NH = 16
HD = 64
N_CORES = 8
HEADS_PER_CORE = 4
DPC = HEADS_PER_CORE * HD  # 256 projection rows per core
P = 128
KC = D // P  # 8 contraction chunks
SC = S // P  # 16 s2 chunks
SQ = 512  # s1 quarter width
NSQ = S // SQ  # 4

_NC_CACHE = {}


def build_nc():
    if "nc" in _NC_CACHE:
        return _NC_CACHE["nc"]
    import concourse.bass as bass
    import concourse.mybir as mybir
    import concourse.tile as tile
    from concourse import bacc

    BF16 = mybir.dt.bfloat16
    F32 = mybir.dt.float32
    Act = mybir.ActivationFunctionType
    ts = bass.ts

    nc = bacc.Bacc(None, target_bir_lowering=False, debug=False)
    xT_d = nc.declare_dram_parameter("xT", [D, S], BF16, isOutput=False)
    wqT_d = nc.declare_dram_parameter("wqT", [D, DPC], BF16, isOutput=False)
    wvT_d = nc.declare_dram_parameter("wvT", [D, DPC], BF16, isOutput=False)
    bq_d = nc.declare_dram_parameter("bq", [DPC, 1], F32, isOutput=False)
    bv_d = nc.declare_dram_parameter("bv", [DPC, 1], F32, isOutput=False)
    bvr_d = nc.declare_dram_parameter("bv_row", [1, DPC], BF16, isOutput=False)
    out_d = nc.declare_dram_parameter("out", [DPC, S], F32, isOutput=True)

    with tile.TileContext(nc) as tc:
        with (
            tc.tile_pool(name="persist", bufs=1) as persist,
            tc.tile_pool(name="pt", bufs=3) as pt_pool,
            tc.tile_pool(name="epi", bufs=2) as epi_pool,
        ):
            # warm the ACT exp table set at t~0 so the one-time table load
            # overlaps the input DMAs
            warm = persist.tile([1, 8], F32, tag="warm")
            nc.vector.memset(warm[:], 0.0)
            nc.scalar.activation(warm[:], warm[:], Act.Exp, scale=1.0)

            # ---- input loads (k-interleaved so matmuls start early) ---------
            xt_sb = [
                persist.tile([P, S], BF16, name=f"xt{k}", tag=f"xt{k}")
                for k in range(KC)
            ]
            wq_sb = [
                persist.tile([P, DPC], BF16, name=f"wq{k}", tag=f"wq{k}")
                for k in range(KC)
            ]
            wv_sb = [
                persist.tile([P, DPC], BF16, name=f"wv{k}", tag=f"wv{k}")
                for k in range(KC)
            ]
            bq_sb = [
                persist.tile([P, 1], F32, name=f"bq{m}", tag=f"bq{m}")
                for m in range(2)
            ]
            bv_sb = [
                persist.tile([P, 1], F32, name=f"bv{m}", tag=f"bv{m}")
                for m in range(2)
            ]
            bvr_sb = persist.tile([1, DPC], BF16, tag="bvr")
            ones_row = persist.tile([1, P], BF16, tag="ones")
            nc.vector.memset(ones_row[:], 1.0)
            nc.sync.dma_start(bvr_sb[:], bvr_d[:])
            for m in range(2):
                nc.sync.dma_start(bq_sb[m][:], bq_d[ts(m, P), :])
                nc.sync.dma_start(bv_sb[m][:], bv_d[ts(m, P), :])
            for k in range(KC):
                nc.sync.dma_start(wq_sb[k][:], wqT_d[ts(k, P), :])
                nc.sync.dma_start(wv_sb[k][:], wvT_d[ts(k, P), :])
                nc.sync.dma_start(xt_sb[k][:], xT_d[ts(k, P), :])

            qT_sb = [
                persist.tile([P, S], BF16, name=f"qT{m}", tag=f"qT{m}")
                for m in range(2)
            ]
            kvT_sb = [
                persist.tile([P, S], BF16, name=f"kvT{m}", tag=f"kvT{m}")
                for m in range(2)
            ]
            # v_sb[p][hl][j]: [128, 65] = V chunk j for head 2p+hl, col 64 = 1
            v_sb = [
                [
                    [
                        persist.tile(
                            [P, HD + 1], BF16,
                            name=f"v{p}_{hl}_{j}", tag=f"v{p}_{hl}_{j}",
                        )
                        for j in range(SC)
                    ]
                    for hl in range(2)
                ]
                for p in range(2)
            ]
            for p in range(2):
                for hl in range(2):
                    for j in range(SC):
                        nc.vector.memset(v_sb[p][hl][j][:, HD : HD + 1], 1.0)

            def proj_steps(w_sb, dst, bias, m, psum_pool):
                """Stepped emission of projection m-chunk [128, 2048]: two
                1024-wide psum passes of 8 K-chunks each."""
                for hf in range(2):
                    ps = psum_pool.tile([P, 1024], F32, tag="mi", name="pp")
                    for k in range(KC):
                        for n in range(2):
                            nc.tensor.matmul(
                                ps[:, ts(n, 512)],
                                w_sb[k][:, ts(m, P)],
                                xt_sb[k][:, ts(hf * 2 + n, 512)],
                                start=(k == 0),
                                stop=(k == KC - 1),
                            )
                        if k % 2 == 1:
                            yield
                    nc.vector.tensor_scalar_add(
                        dst[:, ts(hf, 1024)], ps[:], bias[:]
                    )
                    yield

            def vnat_chunk(j, psum_pool):
                """V natural chunk j for all 4 heads: [128, 256] (+bias)."""
                ps = psum_pool.tile([P, 1024], F32, tag="mi", name="vn")
                for k in range(KC):
                    nc.tensor.matmul(
                        ps[:, 0:DPC],
                        xt_sb[k][:, ts(j, P)],
                        wv_sb[k][:],
                        start=(k == 0),
                        stop=False,
                    )
                nc.tensor.matmul(
                    ps[:, 0:DPC], ones_row[:], bvr_sb[:], start=False, stop=True
                )
                for p in range(2):
                    for hl in range(2):
                        h = 2 * p + hl
                        nc.vector.tensor_copy(
                            v_sb[p][hl][j][:, 0:HD], ps[:, ts(h, HD)]
                        )

            # ---- prologue: m=0 projections, Q and KV interleaved per k ------
            with tc.tile_pool(name="psum_pro", bufs=4, space="PSUM") as psum_pro:
                pq = [psum_pro.tile([P, 1024], F32, tag="pro", name=f"pq{h}")
                      for h in range(2)]
                pv = [psum_pro.tile([P, 1024], F32, tag="pro", name=f"pv{h}")
                      for h in range(2)]
                for k in range(KC):
                    for hf in range(2):
                        for n in range(2):
                            nc.tensor.matmul(
                                pq[hf][:, ts(n, 512)],
                                wq_sb[k][:, 0:P],
                                xt_sb[k][:, ts(hf * 2 + n, 512)],
                                start=(k == 0),
                                stop=(k == KC - 1),
                            )
                    for hf in range(2):
                        for n in range(2):
                            nc.tensor.matmul(
                                pv[hf][:, ts(n, 512)],
                                wv_sb[k][:, 0:P],
                                xt_sb[k][:, ts(hf * 2 + n, 512)],
                                start=(k == 0),
                                stop=(k == KC - 1),
                            )
                for hf in range(2):
                    nc.vector.tensor_scalar_add(
                        qT_sb[0][:, ts(hf, 1024)], pq[hf][:], bq_sb[0][:]
                    )
                    nc.vector.tensor_scalar_add(
                        kvT_sb[0][:, ts(hf, 1024)], pv[hf][:], bv_sb[0][:]
                    )

            # ---- attention ---------------------------------------------------
            with (
                tc.tile_pool(name="psum_sc", bufs=2, space="PSUM") as psum_sc,
                tc.tile_pool(name="psum_at", bufs=2, space="PSUM") as psum_at,
                tc.tile_pool(name="psum_mi", bufs=1, space="PSUM") as psum_mi,
            ):
                def vnat_steps():
                    for j in range(1, SC):
                        vnat_chunk(j, psum_mi)
                        yield

                vnat_chunk(0, psum_mi)
                fillers = {
                    (0, 0): [vnat_steps()],
                    (0, 1): [proj_steps(wv_sb, kvT_sb[1], bv_sb[1], 1, psum_mi)],
                    (0, 2): [proj_steps(wq_sb, qT_sb[1], bq_sb[1], 1, psum_mi)],
                }

                for p in range(2):
                    for q in range(NSQ):
                        gens = fillers.get((p, q), [])
                        at = [
                            psum_at.tile([HD + 1, SQ], F32, tag="at", name="at")
                            for _ in range(2)
                        ]
                        for j in range(SC):
                            sc = psum_sc.tile([P, 1024], F32, tag="sc", name="sc")
                            for hl in range(2):
                                nc.tensor.matmul(
                                    sc[:, ts(hl, SQ)],
                                    kvT_sb[p][hl * HD : (hl + 1) * HD, ts(j, P)],
                                    qT_sb[p][hl * HD : (hl + 1) * HD, ts(q, SQ)],
                                    start=True,
                                    stop=True,
                                    tile_position=(hl * HD, 0),
                                )
                            pt = pt_pool.tile([P, 1024], BF16, tag="pt", name="pt")
                            nc.scalar.activation(pt[:], sc[:], Act.Exp, scale=0.125)
                            for hl in range(2):
                                nc.tensor.matmul(
                                    at[hl][:],
                                    v_sb[p][hl][j][:],
                                    pt[:, ts(hl, SQ)],
                                    start=(j == 0),
                                    stop=(j == SC - 1),
                                )
                            # filler work (Vn chunks, m=1 projections)
                            steps = 0
                            while gens and steps < 2:
                                try:
                                    next(gens[0])
                                    steps += 1
                                except StopIteration:
                                    gens.pop(0)
                        # epilogue per head
                        for hl in range(2):
                            head = 2 * p + hl
                            asb = epi_pool.tile([HD, SQ], F32, tag="asb", name="asb")
                            nc.vector.tensor_copy(asb[:], at[hl][0:HD, :])
                            # partition_broadcast reads the tensor's partition 0
                            # regardless of AP offset: stage the denom row in a
                            # dedicated p0 tile first.
                            dr = epi_pool.tile([1, SQ], F32, tag="dr", name="dr")
                            nc.vector.tensor_copy(dr[:], at[hl][HD : HD + 1, :])
                            bc = epi_pool.tile([HD, SQ], F32, tag="bc", name="bc")
                            nc.gpsimd.partition_broadcast(bc[:], dr[:])
                            rc = epi_pool.tile([HD, SQ], F32, tag="rc", name="rc")
                            nc.vector.reciprocal_approx_fast(rc[:], bc[:])
                            ot = epi_pool.tile([HD, SQ], F32, tag="ot", name="ot")
                            nc.vector.tensor_mul(ot[:], asb[:], rc[:])
                            nc.gpsimd.dma_start(
                                out_d[ts(head, HD), ts(q, SQ)], ot[:]
                            )

    nc.compile()
    _NC_CACHE["nc"] = nc
    return nc


def shard_inputs(x, Wq, bq, Wv, bv):
    bf16 = ml_dtypes.bfloat16
    x = np.asarray(x, dtype=np.float32)
    Wq = np.asarray(Wq, dtype=np.float32)
    bq = np.asarray(bq, dtype=np.float32)
    Wv = np.asarray(Wv, dtype=np.float32)
    bv = np.asarray(bv, dtype=np.float32)
    in_maps = []
    xT = [np.ascontiguousarray(x[b].T).astype(bf16) for b in range(B)]
    for c in range(N_CORES):
        b, g = divmod(c, N_CORES // B)
        heads = [HEADS_PER_CORE * g + hl for hl in range(HEADS_PER_CORE)]
        perm = np.array([i * NH + h for h in heads for i in range(HD)])
        in_maps.append(
            {
                "xT": xT[b],
                "wqT": np.ascontiguousarray(Wq[perm, :].T).astype(bf16),
                "wvT": np.ascontiguousarray(Wv[perm, :].T).astype(bf16),
                "bq": np.ascontiguousarray(bq[perm].reshape(DPC, 1)),
                "bv": np.ascontiguousarray(bv[perm].reshape(DPC, 1)),
                "bv_row": np.ascontiguousarray(bv[perm].reshape(1, DPC)).astype(bf16),
            }
        )
    return in_maps


def assemble(results):
    out = np.empty((B, S, D), dtype=np.float32)
    for c in range(N_CORES):
        b, g = divmod(c, N_CORES // B)
        out[b][:, g * DPC : (g + 1) * DPC] = results[c]["out"].T
    return out


def kernel(x, Wq, bq, Wv, bv):
    from concourse.bass_utils import run_bass_kernel_spmd

    nc = build_nc()
    in_maps = shard_inputs(x, Wq, bq, Wv, bv)
    res = run_bass_kernel_spmd(nc, in_maps, core_ids=list(range(N_CORES)))
    return assemble(res.results)


if __name__ == "__main__":
    rng = np.random.default_rng(0)
    inputs = {
        "x": rng.standard_normal((B, S, D), dtype=np.float32),
        "Wq": (rng.standard_normal((D, D), dtype=np.float32) / 32.0),
        "bq": rng.standard_normal(D, dtype=np.float32) * 0.02,
        "Wv": (rng.standard_normal((D, D), dtype=np.float32) / 32.0),
        "bv": rng.standard_normal(D, dtype=np.float32) * 0.02,
    }
    out = kernel(**inputs)
    print("kernel ran, out shape:", out.shape)


# revision 16
# speedup vs baseline: 1.6039x; 1.1109x over previous
"""Trainium2 Bass kernel for nn_Attention_70136815943694.

Attention with the reference's source bug preserved (K uses the V
projection). x:[2,2048,1024], 16 heads x 64 dim. Sharded over 8
NeuronCores as (batch x head-group): core c handles batch c//4 and
heads [4*(c%4) .. 4*(c%4)+3]. Each core's output slice is independent,
so there are no collectives; the host shards inputs and reassembles.

Per-core device pipeline (d-major layouts):
  QT = wqT.T @ xT (+bq)     [256, 2048] bf16   (DVE evac w/ bias)
  KVT = wvT.T @ xT (+bv)    [256, 2048] bf16
  V   = PE-transpose of KVT chunks (bias already included)
  per head-pair p, s1 quarter q (512 wide), s2 chunk j (128):
    scores: two K=64 matmuls row-packed via tile_position (0,0)/(64,0)
    PT = exp(0.125*scores) on ACT, one [128,1024] op for both heads
    atH += [V_h | 1].T @ PT_h   [65, 512] psum, row 64 = softmax denom
  epilogue: gpsimd partition_broadcast denom, DVE fast reciprocal, mul
The attention phase is ACT(exp)-bound; all projection/V work beyond the
minimal prologue (first halves of QT/KVT m=0) is interleaved into the
attention groups as PE filler so the exp stream starts ~as soon as the
input DMAs land and never starves.
"""
import numpy as np
import ml_dtypes

B = 2
S = 2048
D = 1024
NH = 16
HD = 64
N_CORES = 8
HEADS_PER_CORE = 4
DPC = HEADS_PER_CORE * HD  # 256 projection rows per core
P = 128
KC = D // P  # 8 contraction chunks
SC = S // P  # 16 s2 chunks
SQ = 512  # s1 quarter width
NSQ = S // SQ  # 4

_NC_CACHE = {}


def build_nc():
    if "nc" in _NC_CACHE:
        return _NC_CACHE["nc"]
    import concourse.bass as bass
    import concourse.mybir as mybir
    import concourse.tile as tile
    from concourse import bacc
    from concourse.masks import make_identity

    BF16 = mybir.dt.bfloat16
    F32 = mybir.dt.float32
    Act = mybir.ActivationFunctionType
    ts = bass.ts

    nc = bacc.Bacc(None, target_bir_lowering=False, debug=False)
    xT_d = nc.declare_dram_parameter("xT", [D, S], BF16, isOutput=False)
    wqT_d = nc.declare_dram_parameter("wqT", [D, DPC], BF16, isOutput=False)
    wvT_d = nc.declare_dram_parameter("wvT", [D, DPC], BF16, isOutput=False)
    bq_d = nc.declare_dram_parameter("bq", [DPC, 1], F32, isOutput=False)
    bv_d = nc.declare_dram_parameter("bv", [DPC, 1], F32, isOutput=False)
    out_d = nc.declare_dram_parameter("out", [DPC, S], F32, isOutput=True)

    with tile.TileContext(nc) as tc:
        with (
            tc.tile_pool(name="persist", bufs=1) as persist,
            tc.tile_pool(name="pt", bufs=6) as pt_pool,
            tc.tile_pool(name="epi", bufs=2) as epi_pool,
        ):
            # warm the ACT exp table set at t~0 so the one-time table load
            # overlaps the input DMAs
            warm = persist.tile([1, 8], F32, tag="warm")
            nc.vector.memset(warm[:], 0.0)
            nc.scalar.activation(warm[:], warm[:], Act.Exp, scale=1.0)

            ident = persist.tile([P, P], BF16, tag="ident")
            make_identity(nc, ident[:])

            # ---- input loads: xT on the sync HWDGE queue, weights/biases on
            # the gpsimd SWDGE queue so the issue streams run in parallel.
            xt_sb = [
                persist.tile([P, S], BF16, name=f"xt{k}", tag=f"xt{k}")
                for k in range(KC)
            ]
            wq_sb = [
                persist.tile([P, DPC], BF16, name=f"wq{k}", tag=f"wq{k}")
                for k in range(KC)
            ]
            wv_sb = [
                persist.tile([P, DPC], BF16, name=f"wv{k}", tag=f"wv{k}")
                for k in range(KC)
            ]
            bq_sb = [
                persist.tile([P, 1], F32, name=f"bq{m}", tag=f"bq{m}")
                for m in range(2)
            ]
            bv_sb = [
                persist.tile([P, 1], F32, name=f"bv{m}", tag=f"bv{m}")
                for m in range(2)
            ]
            for k in range(KC):
                nc.sync.dma_start(xt_sb[k][:], xT_d[ts(k, P), :])
            for k in range(KC):
                nc.gpsimd.dma_start(wq_sb[k][:], wqT_d[ts(k, P), :])
                nc.gpsimd.dma_start(wv_sb[k][:], wvT_d[ts(k, P), :])
            for m in range(2):
                nc.gpsimd.dma_start(bq_sb[m][:], bq_d[ts(m, P), :])
                nc.gpsimd.dma_start(bv_sb[m][:], bv_d[ts(m, P), :])

            qT_sb = [
                persist.tile([P, S], BF16, name=f"qT{m}", tag=f"qT{m}")
                for m in range(2)
            ]
            kvT_sb = [
                persist.tile([P, S], BF16, name=f"kvT{m}", tag=f"kvT{m}")
                for m in range(2)
            ]
            # v_sb[p][hl][j]: [128, 65] = V chunk j for head 2p+hl, col 64 = 1
            v_sb = [
                [
                    [
                        persist.tile(
                            [P, HD + 1], BF16,
                            name=f"v{p}_{hl}_{j}", tag=f"v{p}_{hl}_{j}",
                        )
                        for j in range(SC)
                    ]
                    for hl in range(2)
                ]
                for p in range(2)
            ]
            for p in range(2):
                for hl in range(2):
                    for j in range(SC):
                        nc.vector.memset(v_sb[p][hl][j][:, HD : HD + 1], 1.0)

            def proj_pass(w_sb, dst, bias, m, hf, psum_pool, stepped):
                """One 1024-col half of a projection m-chunk. Generator."""
                ps = psum_pool.tile([P, 1024], F32, tag="mi", name="pp")
                for k in range(KC):
                    for n in range(2):
                        nc.tensor.matmul(
                            ps[:, ts(n, 512)],
                            w_sb[k][:, ts(m, P)],
                            xt_sb[k][:, ts(hf * 2 + n, 512)],
                            start=(k == 0),
                            stop=(k == KC - 1),
                        )
                    if stepped and k % 2 == 1:
                        yield
                nc.vector.tensor_scalar_add(dst[:, ts(hf, 1024)], ps[:], bias[:])
                if stepped:
                    yield

            def vtrans_steps(p, psum_pool, j0=0):
                """PE-transpose KVT chunks into natural-layout V tiles."""
                for j in range(j0, SC):
                    pst = psum_pool.tile(
                        [P, P], BF16, tag="mi", name="vt",
                        padded_shape=[P, 2048],
                    )
                    nc.tensor.transpose(
                        pst[:], kvT_sb[p][:, ts(j, P)], ident[:]
                    )
                    for hl in range(2):
                        nc.vector.tensor_copy(
                            v_sb[p][hl][j][:, 0:HD], pst[:, ts(hl, HD)]
                        )
                    yield

            # ---- prologue: first halves (cols 0:1024) of QT/KVT m=0 --------
            with tc.tile_pool(name="psum_pro", bufs=2, space="PSUM") as psum_pro:
                for gen in (
                    proj_pass(wq_sb, qT_sb[0], bq_sb[0], 0, 0, psum_pro, False),
                    proj_pass(wv_sb, kvT_sb[0], bv_sb[0], 0, 0, psum_pro, False),
                ):
                    for _ in gen:
                        pass

            # ---- attention ---------------------------------------------------
            with (
                tc.tile_pool(name="psum_sc", bufs=2, space="PSUM") as psum_sc,
                tc.tile_pool(name="psum_at", bufs=2, space="PSUM") as psum_at,
                tc.tile_pool(name="psum_mi", bufs=1, space="PSUM") as psum_mi,
            ):
                # Preseed the first two V chunks so (0,0)'s first attnT
                # matmuls have emitted writers (Tile deps follow trace order).
                for _ in vtrans_steps(0, psum_mi, j0=0):
                    break  # emits chunk 0 only, then stop
                vt0_rest = vtrans_steps(0, psum_mi, j0=1)
                next(vt0_rest)  # chunk 1
                fillers = {
                    # (0,0): kvT[0] hf1 must be emitted before vtrans chunks
                    # j>=8 read it (round-robin: proj done by iter ~4, vtrans
                    # chunk 2+i at iter i).
                    (0, 0): [
                        vt0_rest,
                        proj_pass(wv_sb, kvT_sb[0], bv_sb[0], 0, 1, psum_mi, True),
                    ],
                    (0, 1): [
                        proj_pass(wq_sb, qT_sb[0], bq_sb[0], 0, 1, psum_mi, True),
                        proj_pass(wv_sb, kvT_sb[1], bv_sb[1], 1, 0, psum_mi, True),
                    ],
                    (0, 2): [
                        proj_pass(wv_sb, kvT_sb[1], bv_sb[1], 1, 1, psum_mi, True),
                        proj_pass(wq_sb, qT_sb[1], bq_sb[1], 1, 0, psum_mi, True),
                    ],
                    (0, 3): [
                        vtrans_steps(1, psum_mi),
                        proj_pass(wq_sb, qT_sb[1], bq_sb[1], 1, 1, psum_mi, True),
                    ],
                }

                for p in range(2):
                    for q in range(NSQ):
                        gens = fillers.get((p, q), [])
                        at = [
                            psum_at.tile([HD + 1, SQ], F32, tag="at", name="at")
                            for _ in range(2)
                        ]
                        for j in range(SC):
                            sc = psum_sc.tile([P, 1024], F32, tag="sc", name="sc")
                            for hl in range(2):
                                nc.tensor.matmul(
                                    sc[:, ts(hl, SQ)],
                                    kvT_sb[p][hl * HD : (hl + 1) * HD, ts(j, P)],
                                    qT_sb[p][hl * HD : (hl + 1) * HD, ts(q, SQ)],
                                    start=True,
                                    stop=True,
                                    tile_position=(hl * HD, 0),
                                )
                            pt = pt_pool.tile([P, 1024], BF16, tag="pt", name="pt")
                            nc.scalar.activation(pt[:], sc[:], Act.Exp, scale=0.125)
                            # filler work (remaining projections, V
                            # transposes), round-robin so every producer's
                            # writes are emitted before their readers
                            for _ in range(2):
                                if not gens:
                                    break
                                g = gens.pop(0)
                                try:
                                    next(g)
                                    gens.append(g)
                                except StopIteration:
                                    pass
                            for hl in range(2):
                                nc.tensor.matmul(
                                    at[hl][:],
                                    v_sb[p][hl][j][:],
                                    pt[:, ts(hl, SQ)],
                                    start=(j == 0),
                                    stop=(j == SC - 1),
                                )
                        # epilogue per head
                        for hl in range(2):
                            head = 2 * p + hl
                            asb = epi_pool.tile([HD, SQ], F32, tag="asb", name="asb")
                            nc.vector.tensor_copy(asb[:], at[hl][0:HD, :])
                            # partition_broadcast reads the tensor's partition 0
                            # regardless of AP offset: stage the denom row in a
                            # dedicated p0 tile first.
                            dr = epi_pool.tile([1, SQ], F32, tag="dr", name="dr")
                            nc.vector.tensor_copy(dr[:], at[hl][HD : HD + 1, :])
                            bc = epi_pool.tile([HD, SQ], F32, tag="bc", name="bc")
                            nc.gpsimd.partition_broadcast(bc[:], dr[:])
                            rc = epi_pool.tile([HD, SQ], F32, tag="rc", name="rc")
                            nc.vector.reciprocal_approx_fast(rc[:], bc[:])
                            ot = epi_pool.tile([HD, SQ], F32, tag="ot", name="ot")
                            nc.vector.tensor_mul(ot[:], asb[:], rc[:])
                            nc.gpsimd.dma_start(
                                out_d[ts(head, HD), ts(q, SQ)], ot[:]
                            )

    nc.compile()
    _NC_CACHE["nc"] = nc
    return nc


def shard_inputs(x, Wq, bq, Wv, bv):
    bf16 = ml_dtypes.bfloat16
    x = np.asarray(x, dtype=np.float32)
    Wq = np.asarray(Wq, dtype=np.float32)
    bq = np.asarray(bq, dtype=np.float32)
    Wv = np.asarray(Wv, dtype=np.float32)
    bv = np.asarray(bv, dtype=np.float32)
    in_maps = []
    xT = [np.ascontiguousarray(x[b].T).astype(bf16) for b in range(B)]
    for c in range(N_CORES):
        b, g = divmod(c, N_CORES // B)
        heads = [HEADS_PER_CORE * g + hl for hl in range(HEADS_PER_CORE)]
        perm = np.array([i * NH + h for h in heads for i in range(HD)])
        in_maps.append(
            {
                "xT": xT[b],
                "wqT": np.ascontiguousarray(Wq[perm, :].T).astype(bf16),
                "wvT": np.ascontiguousarray(Wv[perm, :].T).astype(bf16),
                "bq": np.ascontiguousarray(bq[perm].reshape(DPC, 1)),
                "bv": np.ascontiguousarray(bv[perm].reshape(DPC, 1)),
            }
        )
    return in_maps


def assemble(results):
    out = np.empty((B, S, D), dtype=np.float32)
    for c in range(N_CORES):
        b, g = divmod(c, N_CORES // B)
        out[b][:, g * DPC : (g + 1) * DPC] = results[c]["out"].T
    return out


def kernel(x, Wq, bq, Wv, bv):
    from concourse.bass_utils import run_bass_kernel_spmd

    nc = build_nc()
    in_maps = shard_inputs(x, Wq, bq, Wv, bv)
    res = run_bass_kernel_spmd(nc, in_maps, core_ids=list(range(N_CORES)))
    return assemble(res.results)


if __name__ == "__main__":
    rng = np.random.default_rng(0)
    inputs = {
        "x": rng.standard_normal((B, S, D), dtype=np.float32),
        "Wq": (rng.standard_normal((D, D), dtype=np.float32) / 32.0),
        "bq": rng.standard_normal(D, dtype=np.float32) * 0.02,
        "Wv": (rng.standard_normal((D, D), dtype=np.float32) / 32.0),
        "bv": rng.standard_normal(D, dtype=np.float32) * 0.02,
    }
    out = kernel(**inputs)
    print("kernel ran, out shape:", out.shape)


# revision 17
# speedup vs baseline: 1.6255x; 1.0135x over previous
"""Trainium2 Bass kernel for nn_Attention_70136815943694.

Attention with the reference's source bug preserved (K uses the V
projection). x:[2,2048,1024], 16 heads x 64 dim. Sharded over 8
NeuronCores as (batch x head-group): core c handles batch c//4 and
heads [4*(c%4) .. 4*(c%4)+3]. Each core's output slice is independent,
so there are no collectives; the host shards inputs and reassembles.

Per-core device pipeline (d-major layouts):
  QT = wqT.T @ xT (+bq)     [256, 2048] bf16   (DVE evac w/ bias)
  KVT = wvT.T @ xT (+bv)    [256, 2048] bf16
  V   = PE-transpose of KVT chunks (bias already included)
  per head-pair p, s1 quarter q (512 wide), s2 chunk j (128):
    scores: two K=64 matmuls row-packed via tile_position (0,0)/(64,0)
    PT = exp(0.125*scores) on ACT, one [128,1024] op for both heads
    atH += [V_h | 1].T @ PT_h   [65, 512] psum, row 64 = softmax denom
  epilogue: gpsimd partition_broadcast denom, DVE fast reciprocal, mul
The attention phase is ACT(exp)-bound; all projection/V work beyond the
minimal prologue (first halves of QT/KVT m=0) is interleaved into the
attention groups as PE filler so the exp stream starts ~as soon as the
input DMAs land and never starves.
"""
import numpy as np
import ml_dtypes

B = 2
S = 2048
D = 1024
NH = 16
HD = 64
N_CORES = 8
HEADS_PER_CORE = 4
DPC = HEADS_PER_CORE * HD  # 256 projection rows per core
P = 128
KC = D // P  # 8 contraction chunks
SC = S // P  # 16 s2 chunks
SQ = 512  # s1 quarter width
NSQ = S // SQ  # 4

_NC_CACHE = {}


def build_nc():
    if "nc" in _NC_CACHE:
        return _NC_CACHE["nc"]
    import concourse.bass as bass
    import concourse.mybir as mybir
    import concourse.tile as tile
    from concourse import bacc
    from concourse.masks import make_identity

    BF16 = mybir.dt.bfloat16
    F32 = mybir.dt.float32
    Act = mybir.ActivationFunctionType
    ts = bass.ts

    nc = bacc.Bacc(None, target_bir_lowering=False, debug=False)
    xT_d = nc.declare_dram_parameter("xT", [D, S], BF16, isOutput=False)
    wqT_d = nc.declare_dram_parameter("wqT", [D, DPC], BF16, isOutput=False)
    wvT_d = nc.declare_dram_parameter("wvT", [D, DPC], BF16, isOutput=False)
    bq_d = nc.declare_dram_parameter("bq", [DPC, 1], F32, isOutput=False)
    bv_d = nc.declare_dram_parameter("bv", [DPC, 1], F32, isOutput=False)
    out_d = nc.declare_dram_parameter("out", [DPC, S], F32, isOutput=True)

    with tile.TileContext(nc) as tc:
        with (
            tc.tile_pool(name="persist", bufs=1) as persist,
            tc.tile_pool(name="pt", bufs=6) as pt_pool,
            tc.tile_pool(name="epi", bufs=2) as epi_pool,
        ):
            # warm the ACT exp table set at t~0 so the one-time table load
            # overlaps the input DMAs
            warm = persist.tile([1, 8], F32, tag="warm")
            nc.vector.memset(warm[:], 0.0)
            nc.scalar.activation(warm[:], warm[:], Act.Exp, scale=1.0)

            ident = persist.tile([P, P], BF16, tag="ident")
            make_identity(nc, ident[:])

            # ---- input loads: xT on the sync HWDGE queue, weights/biases on
            # the gpsimd SWDGE queue so the issue streams run in parallel.
            xt_sb = [
                persist.tile([P, S], BF16, name=f"xt{k}", tag=f"xt{k}")
                for k in range(KC)
            ]
            wq_sb = [
                persist.tile([P, DPC], BF16, name=f"wq{k}", tag=f"wq{k}")
                for k in range(KC)
            ]
            wv_sb = [
                persist.tile([P, DPC], BF16, name=f"wv{k}", tag=f"wv{k}")
                for k in range(KC)
            ]
            bq_sb = [
                persist.tile([P, 1], F32, name=f"bq{m}", tag=f"bq{m}")
                for m in range(2)
            ]
            bv_sb = [
                persist.tile([P, 1], F32, name=f"bv{m}", tag=f"bv{m}")
                for m in range(2)
            ]
            for k in range(KC):
                nc.sync.dma_start(xt_sb[k][:], xT_d[ts(k, P), :])
            for k in range(KC):
                nc.gpsimd.dma_start(wq_sb[k][:], wqT_d[ts(k, P), :])
                nc.gpsimd.dma_start(wv_sb[k][:], wvT_d[ts(k, P), :])
            for m in range(2):
                nc.gpsimd.dma_start(bq_sb[m][:], bq_d[ts(m, P), :])
                nc.gpsimd.dma_start(bv_sb[m][:], bv_d[ts(m, P), :])

            qT_sb = [
                persist.tile([P, S], BF16, name=f"qT{m}", tag=f"qT{m}")
                for m in range(2)
            ]
            kvT_sb = [
                persist.tile([P, S], BF16, name=f"kvT{m}", tag=f"kvT{m}")
                for m in range(2)
            ]
            # v_sb[p][hl][j]: [128, 65] = V chunk j for head 2p+hl, col 64 = 1
            v_sb = [
                [
                    [
                        persist.tile(
                            [P, HD + 1], BF16,
                            name=f"v{p}_{hl}_{j}", tag=f"v{p}_{hl}_{j}",
                        )
                        for j in range(SC)
                    ]
                    for hl in range(2)
                ]
                for p in range(2)
            ]
            for p in range(2):
                for hl in range(2):
                    for j in range(SC):
                        nc.vector.memset(v_sb[p][hl][j][:, HD : HD + 1], 1.0)

            def proj512(w_sb, dst, bias, m, c0, psum_pool, stepped):
                """One 512-col slice [c0:c0+512] of a projection m-chunk."""
                ps = psum_pool.tile([P, 512], F32, tag="mi", name="pp")
                nq = c0 // 512
                for k in range(KC):
                    nc.tensor.matmul(
                        ps[:],
                        w_sb[k][:, ts(m, P)],
                        xt_sb[k][:, ts(nq, 512)],
                        start=(k == 0),
                        stop=(k == KC - 1),
                    )
                    if stepped and k % 2 == 1:
                        yield
                nc.vector.tensor_scalar_add(
                    dst[:, ts(nq, 512)], ps[:], bias[:]
                )
                if stepped:
                    yield

            def vtrans_steps(p, psum_pool, j0=0):
                """PE-transpose KVT chunks into natural-layout V tiles."""
                for j in range(j0, SC):
                    pst = psum_pool.tile(
                        [P, P], BF16, tag="mi", name="vt",
                        padded_shape=[P, 1024],
                    )
                    nc.tensor.transpose(
                        pst[:], kvT_sb[p][:, ts(j, P)], ident[:]
                    )
                    for hl in range(2):
                        nc.vector.tensor_copy(
                            v_sb[p][hl][j][:, 0:HD], pst[:, ts(hl, HD)]
                        )
                    if j % 2 == 1:
                        yield

            # ---- prologue: the minimum before exps can flow: qT m0 cols
            # 0:512 ((0,0) scores rhs) and KVT m0 cols 0:1024 (scores lhsT
            # for j<8 plus the first V transposes).
            with tc.tile_pool(name="psum_pro", bufs=4, space="PSUM") as psum_pro:
                for gen in (
                    proj512(wq_sb, qT_sb[0], bq_sb[0], 0, 0, psum_pro, False),
                    proj512(wv_sb, kvT_sb[0], bv_sb[0], 0, 0, psum_pro, False),
                    proj512(wv_sb, kvT_sb[0], bv_sb[0], 0, 512, psum_pro, False),
                ):
                    for _ in gen:
                        pass

            # ---- attention ---------------------------------------------------
            with (
                tc.tile_pool(name="psum_sc", bufs=2, space="PSUM") as psum_sc,
                tc.tile_pool(name="psum_at", bufs=2, space="PSUM") as psum_at,
                tc.tile_pool(name="psum_mi", bufs=2, space="PSUM") as psum_mi,
            ):
                # Preseed the first two V chunks so (0,0)'s first attnT
                # matmuls have emitted writers (Tile deps follow trace order).
                vt0 = vtrans_steps(0, psum_mi, j0=0)
                next(vt0)  # chunks 0,1
                # Filler schedule: each piece lands in the latest group that
                # still meets its consumer's deadline, so no group is
                # overloaded and the ACT exp stream stays dense.
                fillers = {
                    (0, 0): [
                        vt0,  # chunks 2..15 (2/step; attnT-j eats chunk j)
                        proj512(wv_sb, kvT_sb[0], bv_sb[0], 0, 1024, psum_mi, True),
                        proj512(wv_sb, kvT_sb[0], bv_sb[0], 0, 1536, psum_mi, True),
                        proj512(wq_sb, qT_sb[0], bq_sb[0], 0, 512, psum_mi, True),
                    ],
                    (0, 1): [
                        proj512(wq_sb, qT_sb[0], bq_sb[0], 0, 1024, psum_mi, True),
                        proj512(wv_sb, kvT_sb[1], bv_sb[1], 1, 0, psum_mi, True),
                        proj512(wv_sb, kvT_sb[1], bv_sb[1], 1, 512, psum_mi, True),
                    ],
                    (0, 2): [
                        proj512(wq_sb, qT_sb[0], bq_sb[0], 0, 1536, psum_mi, True),
                        proj512(wv_sb, kvT_sb[1], bv_sb[1], 1, 1024, psum_mi, True),
                        proj512(wv_sb, kvT_sb[1], bv_sb[1], 1, 1536, psum_mi, True),
                        proj512(wq_sb, qT_sb[1], bq_sb[1], 1, 0, psum_mi, True),
                    ],
                    (0, 3): [
                        vtrans_steps(1, psum_mi),
                    ],
                    (1, 0): [
                        proj512(wq_sb, qT_sb[1], bq_sb[1], 1, 512, psum_mi, True),
                    ],
                    (1, 1): [
                        proj512(wq_sb, qT_sb[1], bq_sb[1], 1, 1024, psum_mi, True),
                    ],
                    (1, 2): [
                        proj512(wq_sb, qT_sb[1], bq_sb[1], 1, 1536, psum_mi, True),
                    ],
                }

                for p in range(2):
                    for q in range(NSQ):
                        gens = fillers.get((p, q), [])
                        at = [
                            psum_at.tile([HD + 1, SQ], F32, tag="at", name="at")
                            for _ in range(2)
                        ]
                        for j in range(SC):
                            sc = psum_sc.tile([P, 1024], F32, tag="sc", name="sc")
                            for hl in range(2):
                                nc.tensor.matmul(
                                    sc[:, ts(hl, SQ)],
                                    kvT_sb[p][hl * HD : (hl + 1) * HD, ts(j, P)],
                                    qT_sb[p][hl * HD : (hl + 1) * HD, ts(q, SQ)],
                                    start=True,
                                    stop=True,
                                    tile_position=(hl * HD, 0),
                                )
                            pt = pt_pool.tile([P, 1024], BF16, tag="pt", name="pt")
                            nc.scalar.activation(pt[:], sc[:], Act.Exp, scale=0.125)
                            # filler work (remaining projections, V
                            # transposes), round-robin so every producer's
                            # writes are emitted before their readers
                            for _ in range(3):
                                if not gens:
                                    break
                                g = gens.pop(0)
                                try:
                                    next(g)
                                    gens.append(g)
                                except StopIteration:
                                    pass
                            for hl in range(2):
                                nc.tensor.matmul(
                                    at[hl][:],
                                    v_sb[p][hl][j][:],
                                    pt[:, ts(hl, SQ)],
                                    start=(j == 0),
                                    stop=(j == SC - 1),
                                )
                        # epilogue per head
                        for hl in range(2):
                            head = 2 * p + hl
                            asb = epi_pool.tile([HD, SQ], F32, tag="asb", name="asb")
                            nc.vector.tensor_copy(asb[:], at[hl][0:HD, :])
                            # partition_broadcast reads the tensor's partition 0
                            # regardless of AP offset: stage the denom row in a
                            # dedicated p0 tile first.
                            dr = epi_pool.tile([1, SQ], F32, tag="dr", name="dr")
                            nc.vector.tensor_copy(dr[:], at[hl][HD : HD + 1, :])
                            bc = epi_pool.tile([HD, SQ], F32, tag="bc", name="bc")
                            nc.gpsimd.partition_broadcast(bc[:], dr[:])
                            rc = epi_pool.tile([HD, SQ], F32, tag="rc", name="rc")
                            nc.vector.reciprocal_approx_fast(rc[:], bc[:])
                            ot = epi_pool.tile([HD, SQ], F32, tag="ot", name="ot")
                            nc.vector.tensor_mul(ot[:], asb[:], rc[:])
                            nc.gpsimd.dma_start(
                                out_d[ts(head, HD), ts(q, SQ)], ot[:]
                            )

    nc.compile()
    _NC_CACHE["nc"] = nc
    return nc


def shard_inputs(x, Wq, bq, Wv, bv):
    bf16 = ml_dtypes.bfloat16
    x = np.asarray(x, dtype=np.float32)
    Wq = np.asarray(Wq, dtype=np.float32)
    bq = np.asarray(bq, dtype=np.float32)
    Wv = np.asarray(Wv, dtype=np.float32)
    bv = np.asarray(bv, dtype=np.float32)
    in_maps = []
    xT = [np.ascontiguousarray(x[b].T).astype(bf16) for b in range(B)]
    for c in range(N_CORES):
        b, g = divmod(c, N_CORES // B)
        heads = [HEADS_PER_CORE * g + hl for hl in range(HEADS_PER_CORE)]
        perm = np.array([i * NH + h for h in heads for i in range(HD)])
        in_maps.append(
            {
                "xT": xT[b],
                "wqT": np.ascontiguousarray(Wq[perm, :].T).astype(bf16),
                "wvT": np.ascontiguousarray(Wv[perm, :].T).astype(bf16),
                "bq": np.ascontiguousarray(bq[perm].reshape(DPC, 1)),
                "bv": np.ascontiguousarray(bv[perm].reshape(DPC, 1)),
            }
        )
    return in_maps


def assemble(results):
    out = np.empty((B, S, D), dtype=np.float32)
    for c in range(N_CORES):
        b, g = divmod(c, N_CORES // B)
        out[b][:, g * DPC : (g + 1) * DPC] = results[c]["out"].T
    return out


def kernel(x, Wq, bq, Wv, bv):
    from concourse.bass_utils import run_bass_kernel_spmd

    nc = build_nc()
    in_maps = shard_inputs(x, Wq, bq, Wv, bv)
    res = run_bass_kernel_spmd(nc, in_maps, core_ids=list(range(N_CORES)))
    return assemble(res.results)


if __name__ == "__main__":
    rng = np.random.default_rng(0)
    inputs = {
        "x": rng.standard_normal((B, S, D), dtype=np.float32),
        "Wq": (rng.standard_normal((D, D), dtype=np.float32) / 32.0),
        "bq": rng.standard_normal(D, dtype=np.float32) * 0.02,
        "Wv": (rng.standard_normal((D, D), dtype=np.float32) / 32.0),
        "bv": rng.standard_normal(D, dtype=np.float32) * 0.02,
    }
    out = kernel(**inputs)
    print("kernel ran, out shape:", out.shape)


# revision 18
# speedup vs baseline: 1.6817x; 1.0346x over previous
"""Trainium2 Bass kernel for nn_Attention_70136815943694.

Attention with the reference's source bug preserved (K uses the V
projection). x:[2,2048,1024], 16 heads x 64 dim. Sharded over 8
NeuronCores as (batch x head-group): core c handles batch c//4 and
heads [4*(c%4) .. 4*(c%4)+3]. Each core's output slice is independent,
so there are no collectives; the host shards inputs and reassembles.

Per-core device pipeline (d-major layouts):
  QT = wqT.T @ xT (+bq)     [256, 2048] bf16   (DVE evac w/ bias)
  KVT = wvT.T @ xT (+bv)    [256, 2048] bf16
  V   = PE-transpose of KVT chunks (bias already included)
  per head-pair p, s1 quarter q (512 wide), s2 chunk j (128):
    scores: two K=64 matmuls row-packed via tile_position (0,0)/(64,0)
    PT = exp(0.125*scores) on ACT, one [128,1024] op for both heads
    atH += [V_h | 1].T @ PT_h   [65, 512] psum, row 64 = softmax denom
  epilogue: gpsimd partition_broadcast denom, DVE fast reciprocal, mul
The attention phase is ACT(exp)-bound; all projection/V work beyond the
minimal prologue (first halves of QT/KVT m=0) is interleaved into the
attention groups as PE filler so the exp stream starts ~as soon as the
input DMAs land and never starves.
"""
import numpy as np
import ml_dtypes

B = 2
S = 2048
D = 1024
NH = 16
HD = 64
N_CORES = 8
HEADS_PER_CORE = 4
DPC = HEADS_PER_CORE * HD  # 256 projection rows per core
P = 128
KC = D // P  # 8 contraction chunks
SC = S // P  # 16 s2 chunks
SQ = 512  # s1 quarter width
NSQ = S // SQ  # 4

_NC_CACHE = {}


def build_nc():
    if "nc" in _NC_CACHE:
        return _NC_CACHE["nc"]
    import concourse.bass as bass
    import concourse.mybir as mybir
    import concourse.tile as tile
    from concourse import bacc
    from concourse.masks import make_identity

    BF16 = mybir.dt.bfloat16
    F32 = mybir.dt.float32
    Act = mybir.ActivationFunctionType
    ts = bass.ts

    nc = bacc.Bacc(None, target_bir_lowering=False, debug=False)
    xT_d = nc.declare_dram_parameter("xT", [D, S], BF16, isOutput=False)
    wqT_d = nc.declare_dram_parameter("wqT", [D, DPC], BF16, isOutput=False)
    wvT_d = nc.declare_dram_parameter("wvT", [D, DPC], BF16, isOutput=False)
    bq_d = nc.declare_dram_parameter("bq", [DPC, 1], F32, isOutput=False)
    bv_d = nc.declare_dram_parameter("bv", [DPC, 1], F32, isOutput=False)
    out_d = nc.declare_dram_parameter("out", [DPC, S], F32, isOutput=True)

    with tile.TileContext(nc) as tc:
        with (
            tc.tile_pool(name="persist", bufs=1) as persist,
            tc.tile_pool(name="pt", bufs=6) as pt_pool,
            tc.tile_pool(name="epi", bufs=2) as epi_pool,
        ):
            # warm the ACT exp table set at t~0 so the one-time table load
            # overlaps the input DMAs
            warm = persist.tile([1, 8], F32, tag="warm")
            nc.vector.memset(warm[:], 0.0)
            nc.scalar.activation(warm[:], warm[:], Act.Exp, scale=1.0)

            ident = persist.tile([P, P], BF16, tag="ident")
            make_identity(nc, ident[:])

            # ---- input loads: xT on the sync HWDGE queue, weights/biases on
            # the gpsimd SWDGE queue so the issue streams run in parallel.
            xt_sb = [
                persist.tile([P, S], BF16, name=f"xt{k}", tag=f"xt{k}")
                for k in range(KC)
            ]
            wq_sb = [
                persist.tile([P, DPC], BF16, name=f"wq{k}", tag=f"wq{k}")
                for k in range(KC)
            ]
            wv_sb = [
                persist.tile([P, DPC], BF16, name=f"wv{k}", tag=f"wv{k}")
                for k in range(KC)
            ]
            bq_sb = [
                persist.tile([P, 1], F32, name=f"bq{m}", tag=f"bq{m}")
                for m in range(2)
            ]
            bv_sb = [
                persist.tile([P, 1], F32, name=f"bv{m}", tag=f"bv{m}")
                for m in range(2)
            ]
            for k in range(KC):
                nc.sync.dma_start(xt_sb[k][:], xT_d[ts(k, P), :])
            for k in range(KC):
                nc.gpsimd.dma_start(wq_sb[k][:], wqT_d[ts(k, P), :])
                nc.gpsimd.dma_start(wv_sb[k][:], wvT_d[ts(k, P), :])
            for m in range(2):
                nc.gpsimd.dma_start(bq_sb[m][:], bq_d[ts(m, P), :])
                nc.gpsimd.dma_start(bv_sb[m][:], bv_d[ts(m, P), :])

            qT_sb = [
                persist.tile([P, S], BF16, name=f"qT{m}", tag=f"qT{m}")
                for m in range(2)
            ]
            kvT_sb = [
                persist.tile([P, S], BF16, name=f"kvT{m}", tag=f"kvT{m}")
                for m in range(2)
            ]
            # v_sb[p][hl][j]: [128, 65] = V chunk j for head 2p+hl, col 64 = 1
            v_sb = [
                [
                    [
                        persist.tile(
                            [P, HD + 1], BF16,
                            name=f"v{p}_{hl}_{j}", tag=f"v{p}_{hl}_{j}",
                        )
                        for j in range(SC)
                    ]
                    for hl in range(2)
                ]
                for p in range(2)
            ]
            for p in range(2):
                for hl in range(2):
                    for j in range(SC):
                        nc.vector.memset(v_sb[p][hl][j][:, HD : HD + 1], 1.0)

            def proj512(w_sb, dst, bias, m, c0, psum_pool, stepped):
                """One 512-col slice [c0:c0+512] of a projection m-chunk."""
                ps = psum_pool.tile([P, 512], F32, tag="mi", name="pp")
                nq = c0 // 512
                for k in range(KC):
                    nc.tensor.matmul(
                        ps[:],
                        w_sb[k][:, ts(m, P)],
                        xt_sb[k][:, ts(nq, 512)],
                        start=(k == 0),
                        stop=(k == KC - 1),
                    )
                    if stepped and k % 2 == 1:
                        yield
                nc.vector.tensor_scalar_add(
                    dst[:, ts(nq, 512)], ps[:], bias[:]
                )
                if stepped:
                    yield

            def vtrans_steps(p, psum_pool, j0=0):
                """PE-transpose KVT chunks into natural-layout V tiles."""
                for j in range(j0, SC):
                    pst = psum_pool.tile(
                        [P, P], BF16, tag="mi", name="vt",
                        padded_shape=[P, 1024],
                    )
                    nc.tensor.transpose(
                        pst[:], kvT_sb[p][:, ts(j, P)], ident[:]
                    )
                    for hl in range(2):
                        nc.vector.tensor_copy(
                            v_sb[p][hl][j][:, 0:HD], pst[:, ts(hl, HD)]
                        )
                    if j % 2 == 1:
                        yield

            # ---- prologue: the minimum before exps can flow: qT m0 cols
            # 0:512 ((0,0) scores rhs) and KVT m0 cols 0:1024 (scores lhsT
            # for j<8 plus the first V transposes).
            with tc.tile_pool(name="psum_pro", bufs=4, space="PSUM") as psum_pro:
                for gen in (
                    proj512(wq_sb, qT_sb[0], bq_sb[0], 0, 0, psum_pro, False),
                    proj512(wv_sb, kvT_sb[0], bv_sb[0], 0, 0, psum_pro, False),
                    proj512(wv_sb, kvT_sb[0], bv_sb[0], 0, 512, psum_pro, False),
                ):
                    for _ in gen:
                        pass

            # ---- attention ---------------------------------------------------
            with (
                tc.tile_pool(name="psum_sc", bufs=2, space="PSUM") as psum_sc,
                tc.tile_pool(name="psum_at", bufs=2, space="PSUM") as psum_at,
                tc.tile_pool(name="psum_mi", bufs=2, space="PSUM") as psum_mi,
            ):
                # Preseed the first two V chunks so (0,0)'s first attnT
                # matmuls have emitted writers (Tile deps follow trace order).
                vt0 = vtrans_steps(0, psum_mi, j0=0)
                next(vt0)  # chunks 0,1

                def adv(g, n):
                    for _ in range(n):
                        try:
                            next(g)
                        except StopIteration:
                            return

                def fill00():
                    """(0,0) filler with explicit RAW-safe ordering: KVT m0
                    col-slices must be emitted before the V transposes (and
                    scores) that read them; V chunk j before attnT-j."""
                    kv1024 = proj512(wv_sb, kvT_sb[0], bv_sb[0], 0, 1024, psum_mi, True)
                    kv1536 = proj512(wv_sb, kvT_sb[0], bv_sb[0], 0, 1536, psum_mi, True)
                    q512 = proj512(wq_sb, qT_sb[0], bq_sb[0], 0, 512, psum_mi, True)
                    adv(kv1024, 2); yield
                    adv(kv1024, 2); yield
                    adv(kv1024, 1); adv(vt0, 1); yield
                    adv(kv1536, 2); yield
                    adv(kv1536, 2); adv(vt0, 1); yield
                    adv(kv1536, 1); adv(vt0, 1); yield
                    adv(vt0, 1); yield
                    adv(vt0, 1); yield
                    adv(vt0, 1); yield
                    adv(vt0, 1); yield
                    adv(q512, 2); yield
                    adv(q512, 2); yield
                    adv(q512, 1); yield

                # Filler schedule: each piece lands in the latest group that
                # still meets its consumer's deadline, so no group is
                # overloaded and the ACT exp stream stays dense.
                fillers = {
                    (0, 0): [fill00()],
                    (0, 1): [
                        proj512(wq_sb, qT_sb[0], bq_sb[0], 0, 1024, psum_mi, True),
                        proj512(wv_sb, kvT_sb[1], bv_sb[1], 1, 0, psum_mi, True),
                        proj512(wv_sb, kvT_sb[1], bv_sb[1], 1, 512, psum_mi, True),
                    ],
                    (0, 2): [
                        proj512(wq_sb, qT_sb[0], bq_sb[0], 0, 1536, psum_mi, True),
                        proj512(wv_sb, kvT_sb[1], bv_sb[1], 1, 1024, psum_mi, True),
                        proj512(wv_sb, kvT_sb[1], bv_sb[1], 1, 1536, psum_mi, True),
                        proj512(wq_sb, qT_sb[1], bq_sb[1], 1, 0, psum_mi, True),
                    ],
                    (0, 3): [
                        vtrans_steps(1, psum_mi),
                    ],
                    (1, 0): [
                        proj512(wq_sb, qT_sb[1], bq_sb[1], 1, 512, psum_mi, True),
                    ],
                    (1, 1): [
                        proj512(wq_sb, qT_sb[1], bq_sb[1], 1, 1024, psum_mi, True),
                    ],
                    (1, 2): [
                        proj512(wq_sb, qT_sb[1], bq_sb[1], 1, 1536, psum_mi, True),
                    ],
                }

                for p in range(2):
                    for q in range(NSQ):
                        gens = fillers.get((p, q), [])
                        at = [
                            psum_at.tile([HD + 1, SQ], F32, tag="at", name="at")
                            for _ in range(2)
                        ]
                        for j in range(SC):
                            sc = psum_sc.tile([P, 1024], F32, tag="sc", name="sc")
                            for hl in range(2):
                                nc.tensor.matmul(
                                    sc[:, ts(hl, SQ)],
                                    kvT_sb[p][hl * HD : (hl + 1) * HD, ts(j, P)],
                                    qT_sb[p][hl * HD : (hl + 1) * HD, ts(q, SQ)],
                                    start=True,
                                    stop=True,
                                    tile_position=(hl * HD, 0),
                                )
                            pt = pt_pool.tile([P, 1024], BF16, tag="pt", name="pt")
                            nc.scalar.activation(pt[:], sc[:], Act.Exp, scale=0.125)
                            # filler work (remaining projections, V
                            # transposes), round-robin so every producer's
                            # writes are emitted before their readers
                            for _ in range(3):
                                if not gens:
                                    break
                                g = gens.pop(0)
                                try:
                                    next(g)
                                    gens.append(g)
                                except StopIteration:
                                    pass
                            for hl in range(2):
                                nc.tensor.matmul(
                                    at[hl][:],
                                    v_sb[p][hl][j][:],
                                    pt[:, ts(hl, SQ)],
                                    start=(j == 0),
                                    stop=(j == SC - 1),
                                )
                        # epilogue per head
                        for hl in range(2):
                            head = 2 * p + hl
                            asb = epi_pool.tile([HD, SQ], F32, tag="asb", name="asb")
                            nc.vector.tensor_copy(asb[:], at[hl][0:HD, :])
                            # partition_broadcast reads the tensor's partition 0
                            # regardless of AP offset: stage the denom row in a
                            # dedicated p0 tile first.
                            dr = epi_pool.tile([1, SQ], F32, tag="dr", name="dr")
                            nc.vector.tensor_copy(dr[:], at[hl][HD : HD + 1, :])
                            bc = epi_pool.tile([HD, SQ], F32, tag="bc", name="bc")
                            nc.gpsimd.partition_broadcast(bc[:], dr[:])
                            rc = epi_pool.tile([HD, SQ], F32, tag="rc", name="rc")
                            nc.vector.reciprocal_approx_fast(rc[:], bc[:])
                            ot = epi_pool.tile([HD, SQ], F32, tag="ot", name="ot")
                            nc.vector.tensor_mul(ot[:], asb[:], rc[:])
                            nc.gpsimd.dma_start(
                                out_d[ts(head, HD), ts(q, SQ)], ot[:]
                            )

    nc.compile()
    _NC_CACHE["nc"] = nc
    return nc


def shard_inputs(x, Wq, bq, Wv, bv):
    bf16 = ml_dtypes.bfloat16
    x = np.asarray(x, dtype=np.float32)
    Wq = np.asarray(Wq, dtype=np.float32)
    bq = np.asarray(bq, dtype=np.float32)
    Wv = np.asarray(Wv, dtype=np.float32)
    bv = np.asarray(bv, dtype=np.float32)
    in_maps = []
    xT = [np.ascontiguousarray(x[b].T).astype(bf16) for b in range(B)]
    for c in range(N_CORES):
        b, g = divmod(c, N_CORES // B)
        heads = [HEADS_PER_CORE * g + hl for hl in range(HEADS_PER_CORE)]
        perm = np.array([i * NH + h for h in heads for i in range(HD)])
        in_maps.append(
            {
                "xT": xT[b],
                "wqT": np.ascontiguousarray(Wq[perm, :].T).astype(bf16),
                "wvT": np.ascontiguousarray(Wv[perm, :].T).astype(bf16),
                "bq": np.ascontiguousarray(bq[perm].reshape(DPC, 1)),
                "bv": np.ascontiguousarray(bv[perm].reshape(DPC, 1)),
            }
        )
    return in_maps


def assemble(results):
    out = np.empty((B, S, D), dtype=np.float32)
    for c in range(N_CORES):
        b, g = divmod(c, N_CORES // B)
        out[b][:, g * DPC : (g + 1) * DPC] = results[c]["out"].T
    return out


def kernel(x, Wq, bq, Wv, bv):
    from concourse.bass_utils import run_bass_kernel_spmd

    nc = build_nc()
    in_maps = shard_inputs(x, Wq, bq, Wv, bv)
    res = run_bass_kernel_spmd(nc, in_maps, core_ids=list(range(N_CORES)))
    return assemble(res.results)


if __name__ == "__main__":
    rng = np.random.default_rng(0)
    inputs = {
        "x": rng.standard_normal((B, S, D), dtype=np.float32),
        "Wq": (rng.standard_normal((D, D), dtype=np.float32) / 32.0),
        "bq": rng.standard_normal(D, dtype=np.float32) * 0.02,
        "Wv": (rng.standard_normal((D, D), dtype=np.float32) / 32.0),
        "bv": rng.standard_normal(D, dtype=np.float32) * 0.02,
    }
    out = kernel(**inputs)
    print("kernel ran, out shape:", out.shape)


# revision 19
# speedup vs baseline: 1.6858x; 1.0024x over previous
"""Trainium2 Bass kernel for nn_Attention_70136815943694.

Attention with the reference's source bug preserved (K uses the V
projection). x:[2,2048,1024], 16 heads x 64 dim. Sharded over 8
NeuronCores as (batch x head-group): core c handles batch c//4 and
heads [4*(c%4) .. 4*(c%4)+3]. Each core's output slice is independent,
so there are no collectives; the host shards inputs and reassembles.

Per-core device pipeline (d-major layouts):
  QT = wqT.T @ xT (+bq)     [256, 2048] bf16   (DVE evac w/ bias)
  KVT = wvT.T @ xT (+bv)    [256, 2048] bf16
  V   = PE-transpose of KVT chunks (bias already included)
  per head-pair p, s1 quarter q (512 wide), s2 chunk j (128):
    scores: two K=64 matmuls row-packed via tile_position (0,0)/(64,0)
    PT = exp(0.125*scores) on ACT, one [128,1024] op for both heads
    atH += [V_h | 1].T @ PT_h   [65, 512] psum, row 64 = softmax denom
  epilogue: gpsimd partition_broadcast denom, DVE fast reciprocal, mul
The attention phase is ACT(exp)-bound; all projection/V work beyond the
minimal prologue (first halves of QT/KVT m=0) is interleaved into the
attention groups as PE filler so the exp stream starts ~as soon as the
input DMAs land and never starves.
"""
import numpy as np
import ml_dtypes

B = 2
S = 2048
D = 1024
NH = 16
HD = 64
N_CORES = 8
HEADS_PER_CORE = 4
DPC = HEADS_PER_CORE * HD  # 256 projection rows per core
P = 128
KC = D // P  # 8 contraction chunks
SC = S // P  # 16 s2 chunks
SQ = 512  # s1 quarter width
NSQ = S // SQ  # 4

_NC_CACHE = {}


def build_nc():
    if "nc" in _NC_CACHE:
        return _NC_CACHE["nc"]
    import concourse.bass as bass
    import concourse.mybir as mybir
    import concourse.tile as tile
    from concourse import bacc
    from concourse.masks import make_identity

    BF16 = mybir.dt.bfloat16
    F32 = mybir.dt.float32
    Act = mybir.ActivationFunctionType
    ts = bass.ts

    nc = bacc.Bacc(None, target_bir_lowering=False, debug=False)
    xT_d = nc.declare_dram_parameter("xT", [D, S], BF16, isOutput=False)
    wqT_d = nc.declare_dram_parameter("wqT", [D, DPC], BF16, isOutput=False)
    wvT_d = nc.declare_dram_parameter("wvT", [D, DPC], BF16, isOutput=False)
    bq_d = nc.declare_dram_parameter("bq", [DPC, 1], F32, isOutput=False)
    bv_d = nc.declare_dram_parameter("bv", [DPC, 1], F32, isOutput=False)
    out_d = nc.declare_dram_parameter("out", [DPC, S], F32, isOutput=True)

    with tile.TileContext(nc) as tc:
        with (
            tc.tile_pool(name="persist", bufs=1) as persist,
            tc.tile_pool(name="pt", bufs=6) as pt_pool,
            tc.tile_pool(name="epi", bufs=2) as epi_pool,
        ):
            # warm the ACT exp table set at t~0 so the one-time table load
            # overlaps the input DMAs
            warm = persist.tile([1, 8], F32, tag="warm")
            nc.vector.memset(warm[:], 0.0)
            nc.scalar.activation(warm[:], warm[:], Act.Exp, scale=1.0)

            ident = persist.tile([P, P], BF16, tag="ident")
            make_identity(nc, ident[:])

            # ---- input loads: xT on the sync HWDGE queue, weights/biases on
            # the gpsimd SWDGE queue so the issue streams run in parallel.
            xt_sb = [
                persist.tile([P, S], BF16, name=f"xt{k}", tag=f"xt{k}")
                for k in range(KC)
            ]
            wq_sb = [
                persist.tile([P, DPC], BF16, name=f"wq{k}", tag=f"wq{k}")
                for k in range(KC)
            ]
            wv_sb = [
                persist.tile([P, DPC], BF16, name=f"wv{k}", tag=f"wv{k}")
                for k in range(KC)
            ]
            bq_sb = [
                persist.tile([P, 1], F32, name=f"bq{m}", tag=f"bq{m}")
                for m in range(2)
            ]
            bv_sb = [
                persist.tile([P, 1], F32, name=f"bv{m}", tag=f"bv{m}")
                for m in range(2)
            ]
            for k in range(KC):
                nc.sync.dma_start(xt_sb[k][:], xT_d[ts(k, P), :])
            for k in range(KC):
                nc.gpsimd.dma_start(wq_sb[k][:], wqT_d[ts(k, P), :])
                nc.gpsimd.dma_start(wv_sb[k][:], wvT_d[ts(k, P), :])
            for m in range(2):
                nc.gpsimd.dma_start(bq_sb[m][:], bq_d[ts(m, P), :])
                nc.gpsimd.dma_start(bv_sb[m][:], bv_d[ts(m, P), :])

            qT_sb = [
                persist.tile([P, S], BF16, name=f"qT{m}", tag=f"qT{m}")
                for m in range(2)
            ]
            kvT_sb = [
                persist.tile([P, S], BF16, name=f"kvT{m}", tag=f"kvT{m}")
                for m in range(2)
            ]
            # v_sb[p][hl][j]: [128, 65] = V chunk j for head 2p+hl, col 64 = 1
            v_sb = [
                [
                    [
                        persist.tile(
                            [P, HD + 1], BF16,
                            name=f"v{p}_{hl}_{j}", tag=f"v{p}_{hl}_{j}",
                        )
                        for j in range(SC)
                    ]
                    for hl in range(2)
                ]
                for p in range(2)
            ]
            for p in range(2):
                for hl in range(2):
                    for j in range(SC):
                        nc.vector.memset(v_sb[p][hl][j][:, HD : HD + 1], 1.0)

            def proj512(w_sb, dst, bias, m, c0, psum_pool, stepped):
                """One 512-col slice [c0:c0+512] of a projection m-chunk."""
                ps = psum_pool.tile([P, 512], F32, tag="mi", name="pp")
                nq = c0 // 512
                for k in range(KC):
                    nc.tensor.matmul(
                        ps[:],
                        w_sb[k][:, ts(m, P)],
                        xt_sb[k][:, ts(nq, 512)],
                        start=(k == 0),
                        stop=(k == KC - 1),
                    )
                    if stepped and k % 2 == 1:
                        yield
                nc.vector.tensor_scalar_add(
                    dst[:, ts(nq, 512)], ps[:], bias[:]
                )
                if stepped:
                    yield

            def vtrans_steps(p, psum_pool, j0=0):
                """PE-transpose KVT chunks into natural-layout V tiles."""
                for j in range(j0, SC):
                    pst = psum_pool.tile(
                        [P, P], BF16, tag="mi", name="vt",
                        padded_shape=[P, 1024],
                    )
                    nc.tensor.transpose(
                        pst[:], kvT_sb[p][:, ts(j, P)], ident[:]
                    )
                    for hl in range(2):
                        nc.vector.tensor_copy(
                            v_sb[p][hl][j][:, 0:HD], pst[:, ts(hl, HD)]
                        )
                    if j % 2 == 1:
                        yield

            # ---- prologue: the minimum before exps can flow: qT m0 cols
            # 0:512 ((0,0) scores rhs) and KVT m0 cols 0:1024 (scores lhsT
            # for j<8 plus the first V transposes).
            with tc.tile_pool(name="psum_pro", bufs=4, space="PSUM") as psum_pro:
                for gen in (
                    proj512(wq_sb, qT_sb[0], bq_sb[0], 0, 0, psum_pro, False),
                    proj512(wv_sb, kvT_sb[0], bv_sb[0], 0, 0, psum_pro, False),
                ):
                    for _ in gen:
                        pass

            # ---- attention ---------------------------------------------------
            with (
                tc.tile_pool(name="psum_sc", bufs=2, space="PSUM") as psum_sc,
                tc.tile_pool(name="psum_at", bufs=2, space="PSUM") as psum_at,
                tc.tile_pool(name="psum_mi", bufs=2, space="PSUM") as psum_mi,
            ):
                # Preseed the first two V chunks so (0,0)'s first attnT
                # matmuls have emitted writers (Tile deps follow trace order).
                vt0 = vtrans_steps(0, psum_mi, j0=0)
                next(vt0)  # chunks 0,1

                def adv(g, n):
                    for _ in range(n):
                        try:
                            next(g)
                        except StopIteration:
                            return

                def fill00():
                    """(0,0) filler with explicit RAW-safe ordering: KVT m0
                    col-slices must be emitted before the V transposes (and
                    scores) that read them; V chunk j before attnT-j."""
                    kv512 = proj512(wv_sb, kvT_sb[0], bv_sb[0], 0, 512, psum_mi, True)
                    kv1024 = proj512(wv_sb, kvT_sb[0], bv_sb[0], 0, 1024, psum_mi, True)
                    kv1536 = proj512(wv_sb, kvT_sb[0], bv_sb[0], 0, 1536, psum_mi, True)
                    q512 = proj512(wq_sb, qT_sb[0], bq_sb[0], 0, 512, psum_mi, True)
                    adv(kv512, 2); yield
                    adv(kv512, 2); yield
                    adv(kv512, 1); adv(vt0, 1); yield
                    adv(kv1024, 2); yield
                    adv(kv1024, 2); adv(vt0, 1); yield
                    adv(kv1024, 1); adv(vt0, 1); yield
                    adv(kv1536, 2); adv(vt0, 1); yield
                    adv(kv1536, 2); adv(vt0, 1); yield
                    adv(kv1536, 1); adv(vt0, 1); yield
                    adv(vt0, 1); yield
                    adv(q512, 2); yield
                    adv(q512, 2); yield
                    adv(q512, 1); yield

                # Filler schedule: each piece lands in the latest group that
                # still meets its consumer's deadline, so no group is
                # overloaded and the ACT exp stream stays dense.
                fillers = {
                    (0, 0): [fill00()],
                    (0, 1): [
                        proj512(wq_sb, qT_sb[0], bq_sb[0], 0, 1024, psum_mi, True),
                        proj512(wv_sb, kvT_sb[1], bv_sb[1], 1, 0, psum_mi, True),
                        proj512(wv_sb, kvT_sb[1], bv_sb[1], 1, 512, psum_mi, True),
                    ],
                    (0, 2): [
                        proj512(wq_sb, qT_sb[0], bq_sb[0], 0, 1536, psum_mi, True),
                        proj512(wv_sb, kvT_sb[1], bv_sb[1], 1, 1024, psum_mi, True),
                        proj512(wv_sb, kvT_sb[1], bv_sb[1], 1, 1536, psum_mi, True),
                    ],
                    (0, 3): [
                        proj512(wq_sb, qT_sb[1], bq_sb[1], 1, 0, psum_mi, True),
                        vtrans_steps(1, psum_mi),
                    ],
                    (1, 0): [
                        proj512(wq_sb, qT_sb[1], bq_sb[1], 1, 512, psum_mi, True),
                    ],
                    (1, 1): [
                        proj512(wq_sb, qT_sb[1], bq_sb[1], 1, 1024, psum_mi, True),
                    ],
                    (1, 2): [
                        proj512(wq_sb, qT_sb[1], bq_sb[1], 1, 1536, psum_mi, True),
                    ],
                }

                for p in range(2):
                    for q in range(NSQ):
                        gens = fillers.get((p, q), [])
                        at = [
                            psum_at.tile([HD + 1, SQ], F32, tag="at", name="at")
                            for _ in range(2)
                        ]
                        for j in range(SC):
                            sc = psum_sc.tile([P, 1024], F32, tag="sc", name="sc")
                            for hl in range(2):
                                nc.tensor.matmul(
                                    sc[:, ts(hl, SQ)],
                                    kvT_sb[p][hl * HD : (hl + 1) * HD, ts(j, P)],
                                    qT_sb[p][hl * HD : (hl + 1) * HD, ts(q, SQ)],
                                    start=True,
                                    stop=True,
                                    tile_position=(hl * HD, 0),
                                )
                            pt = pt_pool.tile([P, 1024], BF16, tag="pt", name="pt")
                            nc.scalar.activation(pt[:], sc[:], Act.Exp, scale=0.125)
                            # filler work (remaining projections, V
                            # transposes), round-robin so every producer's
                            # writes are emitted before their readers
                            for _ in range(1):
                                if not gens:
                                    break
                                g = gens.pop(0)
                                try:
                                    next(g)
                                    gens.append(g)
                                except StopIteration:
                                    pass
                            for hl in range(2):
                                nc.tensor.matmul(
                                    at[hl][:],
                                    v_sb[p][hl][j][:],
                                    pt[:, ts(hl, SQ)],
                                    start=(j == 0),
                                    stop=(j == SC - 1),
                                )
                        # epilogue per head
                        for hl in range(2):
                            head = 2 * p + hl
                            asb = epi_pool.tile([HD, SQ], F32, tag="asb", name="asb")
                            nc.vector.tensor_copy(asb[:], at[hl][0:HD, :])
                            # partition_broadcast reads the tensor's partition 0
                            # regardless of AP offset: stage the denom row in a
                            # dedicated p0 tile first.
                            dr = epi_pool.tile([1, SQ], F32, tag="dr", name="dr")
                            nc.vector.tensor_copy(dr[:], at[hl][HD : HD + 1, :])
                            bc = epi_pool.tile([HD, SQ], F32, tag="bc", name="bc")
                            nc.gpsimd.partition_broadcast(bc[:], dr[:])
                            rc = epi_pool.tile([HD, SQ], F32, tag="rc", name="rc")
                            nc.vector.reciprocal_approx_fast(rc[:], bc[:])
                            ot = epi_pool.tile([HD, SQ], F32, tag="ot", name="ot")
                            nc.vector.tensor_mul(ot[:], asb[:], rc[:])
                            nc.gpsimd.dma_start(
                                out_d[ts(head, HD), ts(q, SQ)], ot[:]
                            )

    nc.compile()
    _NC_CACHE["nc"] = nc
    return nc


def shard_inputs(x, Wq, bq, Wv, bv):
    bf16 = ml_dtypes.bfloat16
    x = np.asarray(x, dtype=np.float32)
    Wq = np.asarray(Wq, dtype=np.float32)
    bq = np.asarray(bq, dtype=np.float32)
    Wv = np.asarray(Wv, dtype=np.float32)
    bv = np.asarray(bv, dtype=np.float32)
    in_maps = []
    xT = [np.ascontiguousarray(x[b].T).astype(bf16) for b in range(B)]
    for c in range(N_CORES):
        b, g = divmod(c, N_CORES // B)
        heads = [HEADS_PER_CORE * g + hl for hl in range(HEADS_PER_CORE)]
        perm = np.array([i * NH + h for h in heads for i in range(HD)])
        in_maps.append(
            {
                "xT": xT[b],
                "wqT": np.ascontiguousarray(Wq[perm, :].T).astype(bf16),
                "wvT": np.ascontiguousarray(Wv[perm, :].T).astype(bf16),
                "bq": np.ascontiguousarray(bq[perm].reshape(DPC, 1)),
                "bv": np.ascontiguousarray(bv[perm].reshape(DPC, 1)),
            }
        )
    return in_maps


def assemble(results):
    out = np.empty((B, S, D), dtype=np.float32)
    for c in range(N_CORES):
        b, g = divmod(c, N_CORES // B)
        out[b][:, g * DPC : (g + 1) * DPC] = results[c]["out"].T
    return out


def kernel(x, Wq, bq, Wv, bv):
    from concourse.bass_utils import run_bass_kernel_spmd

    nc = build_nc()
    in_maps = shard_inputs(x, Wq, bq, Wv, bv)
    res = run_bass_kernel_spmd(nc, in_maps, core_ids=list(range(N_CORES)))
    return assemble(res.results)


if __name__ == "__main__":
    rng = np.random.default_rng(0)
    inputs = {
        "x": rng.standard_normal((B, S, D), dtype=np.float32),
        "Wq": (rng.standard_normal((D, D), dtype=np.float32) / 32.0),
        "bq": rng.standard_normal(D, dtype=np.float32) * 0.02,
        "Wv": (rng.standard_normal((D, D), dtype=np.float32) / 32.0),
        "bv": rng.standard_normal(D, dtype=np.float32) * 0.02,
    }
    out = kernel(**inputs)
    print("kernel ran, out shape:", out.shape)


# revision 20
# speedup vs baseline: 1.7389x; 1.0315x over previous
"""Trainium2 Bass kernel for nn_Attention_70136815943694.

Attention with the reference's source bug preserved (K uses the V
projection). x:[2,2048,1024], 16 heads x 64 dim. Sharded over 8
NeuronCores as (batch x head-group): core c handles batch c//4 and
heads [4*(c%4) .. 4*(c%4)+3]. Each core's output slice is independent,
so there are no collectives; the host shards inputs and reassembles.

Per-core device pipeline (d-major layouts):
  QT = wqT.T @ xT (+bq)     [256, 2048] bf16   (DVE evac w/ bias)
  KVT = wvT.T @ xT (+bv)    [256, 2048] bf16
  V   = PE-transpose of KVT chunks (bias already included)
  per head-pair p, s1 quarter q (512 wide), s2 chunk j (128):
    scores: two K=64 matmuls row-packed via tile_position (0,0)/(64,0)
    PT = exp(0.125*scores) on ACT, one [128,1024] op for both heads
    atH += [V_h | 1].T @ PT_h   [65, 512] psum, row 64 = softmax denom
  epilogue: gpsimd partition_broadcast denom, DVE fast reciprocal, mul
The attention phase is ACT(exp)-bound; all projection/V work beyond the
minimal prologue (first halves of QT/KVT m=0) is interleaved into the
attention groups as PE filler so the exp stream starts ~as soon as the
input DMAs land and never starves.
"""
import numpy as np
import ml_dtypes

B = 2
S = 2048
D = 1024
NH = 16
HD = 64
N_CORES = 8
HEADS_PER_CORE = 4
DPC = HEADS_PER_CORE * HD  # 256 projection rows per core
P = 128
KC = D // P  # 8 contraction chunks
SC = S // P  # 16 s2 chunks
SQ = 512  # s1 quarter width
NSQ = S // SQ  # 4

_NC_CACHE = {}


def build_nc():
    if "nc" in _NC_CACHE:
        return _NC_CACHE["nc"]
    import concourse.bass as bass
    import concourse.mybir as mybir
    import concourse.tile as tile
    from concourse import bacc
    from concourse.masks import make_identity

    BF16 = mybir.dt.bfloat16
    F32 = mybir.dt.float32
    Act = mybir.ActivationFunctionType
    ts = bass.ts

    nc = bacc.Bacc(None, target_bir_lowering=False, debug=False)
    xT_d = nc.declare_dram_parameter("xT", [D, S], BF16, isOutput=False)
    wqT_d = nc.declare_dram_parameter("wqT", [D, DPC], BF16, isOutput=False)
    wvT_d = nc.declare_dram_parameter("wvT", [D, DPC], BF16, isOutput=False)
    bq_d = nc.declare_dram_parameter("bq", [DPC, 1], F32, isOutput=False)
    bv_d = nc.declare_dram_parameter("bv", [DPC, 1], F32, isOutput=False)
    out_d = nc.declare_dram_parameter("out", [DPC, S], F32, isOutput=True)

    with tile.TileContext(nc) as tc:
        with (
            tc.tile_pool(name="persist", bufs=1) as persist,
            tc.tile_pool(name="pt", bufs=8) as pt_pool,
            tc.tile_pool(name="epi", bufs=2) as epi_pool,
        ):
            # warm the ACT exp table set at t~0 so the one-time table load
            # overlaps the input DMAs
            warm = persist.tile([1, 8], F32, tag="warm")
            nc.vector.memset(warm[:], 0.0)
            nc.scalar.activation(warm[:], warm[:], Act.Exp, scale=1.0)

            ident = persist.tile([P, P], BF16, tag="ident")
            make_identity(nc, ident[:])

            # ---- input loads: xT on the sync HWDGE queue, weights/biases on
            # the gpsimd SWDGE queue so the issue streams run in parallel.
            xt_sb = [
                persist.tile([P, S], BF16, name=f"xt{k}", tag=f"xt{k}")
                for k in range(KC)
            ]
            wq_sb = [
                persist.tile([P, DPC], BF16, name=f"wq{k}", tag=f"wq{k}")
                for k in range(KC)
            ]
            wv_sb = [
                persist.tile([P, DPC], BF16, name=f"wv{k}", tag=f"wv{k}")
                for k in range(KC)
            ]
            bq_sb = [
                persist.tile([P, 1], F32, name=f"bq{m}", tag=f"bq{m}")
                for m in range(2)
            ]
            bv_sb = [
                persist.tile([P, 1], F32, name=f"bv{m}", tag=f"bv{m}")
                for m in range(2)
            ]
            # wq0/wv0 first (small, unblock the first LDWEIGHTS), then xT in
            # column-quarters, column-major: the prologue only reads cols
            # 0:512 of every k-chunk, so its 1MB lands in ~3us instead of
            # waiting for the full 4MB.
            nc.sync.dma_start(wq_sb[0][:], wqT_d[ts(0, P), :])
            nc.sync.dma_start(wv_sb[0][:], wvT_d[ts(0, P), :])
            for cq in range(4):
                for k in range(KC):
                    nc.sync.dma_start(
                        xt_sb[k][:, ts(cq, 512)], xT_d[ts(k, P), ts(cq, 512)]
                    )
            for k in range(1, KC):
                nc.gpsimd.dma_start(wq_sb[k][:], wqT_d[ts(k, P), :])
                nc.gpsimd.dma_start(wv_sb[k][:], wvT_d[ts(k, P), :])
            for m in range(2):
                nc.gpsimd.dma_start(bq_sb[m][:], bq_d[ts(m, P), :])
                nc.gpsimd.dma_start(bv_sb[m][:], bv_d[ts(m, P), :])

            qT_sb = [
                persist.tile([P, S], BF16, name=f"qT{m}", tag=f"qT{m}")
                for m in range(2)
            ]
            kvT_sb = [
                persist.tile([P, S], BF16, name=f"kvT{m}", tag=f"kvT{m}")
                for m in range(2)
            ]
            # v_sb[p][hl][j]: [128, 65] = V chunk j for head 2p+hl, col 64 = 1
            v_sb = [
                [
                    [
                        persist.tile(
                            [P, HD + 1], BF16,
                            name=f"v{p}_{hl}_{j}", tag=f"v{p}_{hl}_{j}",
                        )
                        for j in range(SC)
                    ]
                    for hl in range(2)
                ]
                for p in range(2)
            ]
            for p in range(2):
                for hl in range(2):
                    for j in range(SC):
                        nc.vector.memset(v_sb[p][hl][j][:, HD : HD + 1], 1.0)

            def proj512(w_sb, dst, bias, m, c0, psum_pool, stepped):
                """One 512-col slice [c0:c0+512] of a projection m-chunk."""
                ps = psum_pool.tile([P, 512], F32, tag="mi", name="pp")
                nq = c0 // 512
                for k in range(KC):
                    nc.tensor.matmul(
                        ps[:],
                        w_sb[k][:, ts(m, P)],
                        xt_sb[k][:, ts(nq, 512)],
                        start=(k == 0),
                        stop=(k == KC - 1),
                    )
                    if stepped and k % 2 == 1:
                        yield
                nc.vector.tensor_scalar_add(
                    dst[:, ts(nq, 512)], ps[:], bias[:]
                )
                if stepped:
                    yield

            def vtrans_steps(p, psum_pool, j0=0):
                """PE-transpose KVT chunks into natural-layout V tiles."""
                for j in range(j0, SC):
                    pst = psum_pool.tile(
                        [P, P], BF16, tag="mi", name="vt",
                        padded_shape=[P, 1024],
                    )
                    nc.tensor.transpose(
                        pst[:], kvT_sb[p][:, ts(j, P)], ident[:]
                    )
                    for hl in range(2):
                        nc.vector.tensor_copy(
                            v_sb[p][hl][j][:, 0:HD], pst[:, ts(hl, HD)]
                        )
                    if j % 2 == 1:
                        yield

            # ---- prologue: the minimum before exps can flow: qT m0 cols
            # 0:512 ((0,0) scores rhs) and KVT m0 cols 0:1024 (scores lhsT
            # for j<8 plus the first V transposes).
            with tc.tile_pool(name="psum_pro", bufs=4, space="PSUM") as psum_pro:
                for gen in (
                    proj512(wq_sb, qT_sb[0], bq_sb[0], 0, 0, psum_pro, False),
                    proj512(wv_sb, kvT_sb[0], bv_sb[0], 0, 0, psum_pro, False),
                ):
                    for _ in gen:
                        pass

            # ---- attention ---------------------------------------------------
            with (
                tc.tile_pool(name="psum_sc", bufs=2, space="PSUM") as psum_sc,
                tc.tile_pool(name="psum_at", bufs=2, space="PSUM") as psum_at,
                tc.tile_pool(name="psum_mi", bufs=2, space="PSUM") as psum_mi,
            ):
                # Preseed the first two V chunks so (0,0)'s first attnT
                # matmuls have emitted writers (Tile deps follow trace order).
                vt0 = vtrans_steps(0, psum_mi, j0=0)
                next(vt0)  # chunks 0,1

                def adv(g, n):
                    for _ in range(n):
                        try:
                            next(g)
                        except StopIteration:
                            return

                def fill00():
                    """(0,0) filler with explicit RAW-safe ordering: KVT m0
                    col-slices must be emitted before the V transposes (and
                    scores) that read them; V chunk j before attnT-j."""
                    kv512 = proj512(wv_sb, kvT_sb[0], bv_sb[0], 0, 512, psum_mi, True)
                    kv1024 = proj512(wv_sb, kvT_sb[0], bv_sb[0], 0, 1024, psum_mi, True)
                    kv1536 = proj512(wv_sb, kvT_sb[0], bv_sb[0], 0, 1536, psum_mi, True)
                    q512 = proj512(wq_sb, qT_sb[0], bq_sb[0], 0, 512, psum_mi, True)
                    adv(kv512, 2); yield
                    adv(kv512, 2); yield
                    adv(kv512, 1); adv(vt0, 1); yield
                    adv(kv1024, 2); yield
                    adv(kv1024, 2); adv(vt0, 1); yield
                    adv(kv1024, 1); adv(vt0, 1); yield
                    adv(kv1536, 2); adv(vt0, 1); yield
                    adv(kv1536, 2); adv(vt0, 1); yield
                    adv(kv1536, 1); adv(vt0, 1); yield
                    adv(vt0, 1); yield
                    adv(q512, 2); yield
                    adv(q512, 2); yield
                    adv(q512, 1); yield

                # Filler schedule: each piece lands in the latest group that
                # still meets its consumer's deadline, so no group is
                # overloaded and the ACT exp stream stays dense.
                fillers = {
                    (0, 0): [fill00()],
                    (0, 1): [
                        proj512(wq_sb, qT_sb[0], bq_sb[0], 0, 1024, psum_mi, True),
                        proj512(wv_sb, kvT_sb[1], bv_sb[1], 1, 0, psum_mi, True),
                        proj512(wv_sb, kvT_sb[1], bv_sb[1], 1, 512, psum_mi, True),
                    ],
                    (0, 2): [
                        proj512(wq_sb, qT_sb[0], bq_sb[0], 0, 1536, psum_mi, True),
                        proj512(wv_sb, kvT_sb[1], bv_sb[1], 1, 1024, psum_mi, True),
                        proj512(wv_sb, kvT_sb[1], bv_sb[1], 1, 1536, psum_mi, True),
                    ],
                    (0, 3): [
                        proj512(wq_sb, qT_sb[1], bq_sb[1], 1, 0, psum_mi, True),
                        vtrans_steps(1, psum_mi),
                    ],
                    (1, 0): [
                        proj512(wq_sb, qT_sb[1], bq_sb[1], 1, 512, psum_mi, True),
                    ],
                    (1, 1): [
                        proj512(wq_sb, qT_sb[1], bq_sb[1], 1, 1024, psum_mi, True),
                    ],
                    (1, 2): [
                        proj512(wq_sb, qT_sb[1], bq_sb[1], 1, 1536, psum_mi, True),
                    ],
                }

                for p in range(2):
                    for q in range(NSQ):
                        gens = fillers.get((p, q), [])
                        at = [
                            psum_at.tile([HD + 1, SQ], F32, tag="at", name="at")
                            for _ in range(2)
                        ]
                        for j in range(SC):
                            sc = psum_sc.tile([P, 1024], F32, tag="sc", name="sc")
                            for hl in range(2):
                                nc.tensor.matmul(
                                    sc[:, ts(hl, SQ)],
                                    kvT_sb[p][hl * HD : (hl + 1) * HD, ts(j, P)],
                                    qT_sb[p][hl * HD : (hl + 1) * HD, ts(q, SQ)],
                                    start=True,
                                    stop=True,
                                    tile_position=(hl * HD, 0),
                                )
                            pt = pt_pool.tile([P, 1024], BF16, tag="pt", name="pt")
                            nc.scalar.activation(pt[:], sc[:], Act.Exp, scale=0.125)
                            # filler work (remaining projections, V
                            # transposes), round-robin so every producer's
                            # writes are emitted before their readers
                            for _ in range(1):
                                if not gens:
                                    break
                                g = gens.pop(0)
                                try:
                                    next(g)
                                    gens.append(g)
                                except StopIteration:
                                    pass
                            for hl in range(2):
                                nc.tensor.matmul(
                                    at[hl][:],
                                    v_sb[p][hl][j][:],
                                    pt[:, ts(hl, SQ)],
                                    start=(j == 0),
                                    stop=(j == SC - 1),
                                )
                        # epilogue per head
                        for hl in range(2):
                            head = 2 * p + hl
                            asb = epi_pool.tile([HD, SQ], F32, tag="asb", name="asb")
                            nc.vector.tensor_copy(asb[:], at[hl][0:HD, :])
                            # partition_broadcast reads the tensor's partition 0
                            # regardless of AP offset: stage the denom row in a
                            # dedicated p0 tile first.
                            dr = epi_pool.tile([1, SQ], F32, tag="dr", name="dr")
                            nc.vector.tensor_copy(dr[:], at[hl][HD : HD + 1, :])
                            bc = epi_pool.tile([HD, SQ], F32, tag="bc", name="bc")
                            nc.gpsimd.partition_broadcast(bc[:], dr[:])
                            rc = epi_pool.tile([HD, SQ], F32, tag="rc", name="rc")
                            nc.vector.reciprocal_approx_fast(rc[:], bc[:])
                            ot = epi_pool.tile([HD, SQ], F32, tag="ot", name="ot")
                            nc.vector.tensor_mul(ot[:], asb[:], rc[:])
                            nc.gpsimd.dma_start(
                                out_d[ts(head, HD), ts(q, SQ)], ot[:]
                            )

    nc.compile()
    _NC_CACHE["nc"] = nc
    return nc


def shard_inputs(x, Wq, bq, Wv, bv):
    bf16 = ml_dtypes.bfloat16
    x = np.asarray(x, dtype=np.float32)
    Wq = np.asarray(Wq, dtype=np.float32)
    bq = np.asarray(bq, dtype=np.float32)
    Wv = np.asarray(Wv, dtype=np.float32)
    bv = np.asarray(bv, dtype=np.float32)
    in_maps = []
    xT = [np.ascontiguousarray(x[b].T).astype(bf16) for b in range(B)]
    for c in range(N_CORES):
        b, g = divmod(c, N_CORES // B)
        heads = [HEADS_PER_CORE * g + hl for hl in range(HEADS_PER_CORE)]
        perm = np.array([i * NH + h for h in heads for i in range(HD)])
        in_maps.append(
            {
                "xT": xT[b],
                "wqT": np.ascontiguousarray(Wq[perm, :].T).astype(bf16),
                "wvT": np.ascontiguousarray(Wv[perm, :].T).astype(bf16),
                "bq": np.ascontiguousarray(bq[perm].reshape(DPC, 1)),
                "bv": np.ascontiguousarray(bv[perm].reshape(DPC, 1)),
            }
        )
    return in_maps


def assemble(results):
    out = np.empty((B, S, D), dtype=np.float32)
    for c in range(N_CORES):
        b, g = divmod(c, N_CORES // B)
        out[b][:, g * DPC : (g + 1) * DPC] = results[c]["out"].T
    return out


def kernel(x, Wq, bq, Wv, bv):
    from concourse.bass_utils import run_bass_kernel_spmd

    nc = build_nc()
    in_maps = shard_inputs(x, Wq, bq, Wv, bv)
    res = run_bass_kernel_spmd(nc, in_maps, core_ids=list(range(N_CORES)))
    return assemble(res.results)


if __name__ == "__main__":
    rng = np.random.default_rng(0)
    inputs = {
        "x": rng.standard_normal((B, S, D), dtype=np.float32),
        "Wq": (rng.standard_normal((D, D), dtype=np.float32) / 32.0),
        "bq": rng.standard_normal(D, dtype=np.float32) * 0.02,
        "Wv": (rng.standard_normal((D, D), dtype=np.float32) / 32.0),
        "bv": rng.standard_normal(D, dtype=np.float32) * 0.02,
    }
    out = kernel(**inputs)
    print("kernel ran, out shape:", out.shape)


# revision 21
# speedup vs baseline: 1.7663x; 1.0157x over previous
"""Trainium2 Bass kernel for nn_Attention_70136815943694.

Attention with the reference's source bug preserved (K uses the V
projection). x:[2,2048,1024], 16 heads x 64 dim. Sharded over 8
NeuronCores as (batch x head-group): core c handles batch c//4 and
heads [4*(c%4) .. 4*(c%4)+3]. Each core's output slice is independent,
so there are no collectives; the host shards inputs and reassembles.

Per-core device pipeline (d-major layouts):
  QT = wqT.T @ xT (+bq)     [256, 2048] bf16   (DVE evac w/ bias)
  KVT = wvT.T @ xT (+bv)    [256, 2048] bf16
  V   = PE-transpose of KVT chunks (bias already included)
  per head-pair p, s1 quarter q (512 wide), s2 chunk j (128):
    scores: two K=64 matmuls row-packed via tile_position (0,0)/(64,0)
    PT = exp(0.125*scores) on ACT, one [128,1024] op for both heads
    atH += [V_h | 1].T @ PT_h   [65, 512] psum, row 64 = softmax denom
  epilogue: gpsimd partition_broadcast denom, DVE fast reciprocal, mul
The attention phase is ACT(exp)-bound; all projection/V work beyond the
minimal prologue (first halves of QT/KVT m=0) is interleaved into the
attention groups as PE filler so the exp stream starts ~as soon as the
input DMAs land and never starves.
"""
import numpy as np
import ml_dtypes

B = 2
S = 2048
D = 1024
NH = 16
HD = 64
N_CORES = 8
HEADS_PER_CORE = 4
DPC = HEADS_PER_CORE * HD  # 256 projection rows per core
P = 128
KC = D // P  # 8 contraction chunks
SC = S // P  # 16 s2 chunks
SQ = 512  # s1 quarter width
NSQ = S // SQ  # 4

_NC_CACHE = {}


def build_nc():
    if "nc" in _NC_CACHE:
        return _NC_CACHE["nc"]
    import concourse.bass as bass
    import concourse.mybir as mybir
    import concourse.tile as tile
    from concourse import bacc
    from concourse.masks import make_identity

    BF16 = mybir.dt.bfloat16
    F32 = mybir.dt.float32
    Act = mybir.ActivationFunctionType
    ts = bass.ts

    nc = bacc.Bacc(None, target_bir_lowering=False, debug=False)
    xT_d = nc.declare_dram_parameter("xT", [D, S], BF16, isOutput=False)
    wqT_d = nc.declare_dram_parameter("wqT", [D, DPC], BF16, isOutput=False)
    wvT_d = nc.declare_dram_parameter("wvT", [D, DPC], BF16, isOutput=False)
    bq_d = nc.declare_dram_parameter("bq", [DPC, 1], F32, isOutput=False)
    bv_d = nc.declare_dram_parameter("bv", [DPC, 1], F32, isOutput=False)
    out_d = nc.declare_dram_parameter("out", [DPC, S], F32, isOutput=True)

    with tile.TileContext(nc) as tc:
        with (
            tc.tile_pool(name="persist", bufs=1) as persist,
            tc.tile_pool(name="pt", bufs=8) as pt_pool,
            tc.tile_pool(name="epi", bufs=2) as epi_pool,
        ):
            # warm the ACT exp table set at t~0 so the one-time table load
            # overlaps the input DMAs
            warm = persist.tile([1, 8], F32, tag="warm")
            nc.vector.memset(warm[:], 0.0)
            nc.scalar.activation(warm[:], warm[:], Act.Exp, scale=1.0)

            ident = persist.tile([P, P], BF16, tag="ident")
            make_identity(nc, ident[:])

            # ---- input loads: xT on the sync HWDGE queue, weights/biases on
            # the gpsimd SWDGE queue so the issue streams run in parallel.
            xt_sb = [
                persist.tile([P, S], BF16, name=f"xt{k}", tag=f"xt{k}")
                for k in range(KC)
            ]
            wq_sb = [
                persist.tile([P, DPC], BF16, name=f"wq{k}", tag=f"wq{k}")
                for k in range(KC)
            ]
            wv_sb = [
                persist.tile([P, DPC], BF16, name=f"wv{k}", tag=f"wv{k}")
                for k in range(KC)
            ]
            bq_sb = [
                persist.tile([P, 1], F32, name=f"bq{m}", tag=f"bq{m}")
                for m in range(2)
            ]
            bv_sb = [
                persist.tile([P, 1], F32, name=f"bv{m}", tag=f"bv{m}")
                for m in range(2)
            ]
            # wq0/wv0 first (small, unblock the first LDWEIGHTS), then xT in
            # column-quarters, column-major: the prologue only reads cols
            # 0:512 of every k-chunk, so its 1MB lands in ~3us instead of
            # waiting for the full 4MB.
            nc.sync.dma_start(wq_sb[0][:], wqT_d[ts(0, P), :])
            nc.sync.dma_start(wv_sb[0][:], wvT_d[ts(0, P), :])
            for cq in range(4):
                for k in range(KC):
                    nc.sync.dma_start(
                        xt_sb[k][:, ts(cq, 512)], xT_d[ts(k, P), ts(cq, 512)]
                    )
            for k in range(1, KC):
                nc.gpsimd.dma_start(wq_sb[k][:], wqT_d[ts(k, P), :])
                nc.gpsimd.dma_start(wv_sb[k][:], wvT_d[ts(k, P), :])
            for m in range(2):
                nc.gpsimd.dma_start(bq_sb[m][:], bq_d[ts(m, P), :])
                nc.gpsimd.dma_start(bv_sb[m][:], bv_d[ts(m, P), :])

            qT_sb = [
                persist.tile([P, S], BF16, name=f"qT{m}", tag=f"qT{m}")
                for m in range(2)
            ]
            kvT_sb = [
                persist.tile([P, S], BF16, name=f"kvT{m}", tag=f"kvT{m}")
                for m in range(2)
            ]
            # v_sb[p][hl][j]: [128, 65] = V chunk j for head 2p+hl, col 64 = 1
            v_sb = [
                [
                    [
                        persist.tile(
                            [P, HD + 1], BF16,
                            name=f"v{p}_{hl}_{j}", tag=f"v{p}_{hl}_{j}",
                        )
                        for j in range(SC)
                    ]
                    for hl in range(2)
                ]
                for p in range(2)
            ]
            for p in range(2):
                for hl in range(2):
                    for j in range(SC):
                        nc.vector.memset(v_sb[p][hl][j][:, HD : HD + 1], 1.0)

            def proj512(w_sb, dst, bias, m, c0, psum_pool, stepped, warm=0):
                """One 512-col slice [c0:c0+512] of a projection m-chunk.
                warm>0 emits that many throwaway ident matmuls into the psum
                first (overwritten by the real k0 start=True) to lift the PE
                HAM clock gate during the DMA-bound start."""
                ps = psum_pool.tile([P, 512], F32, tag="mi", name="pp")
                nq = c0 // 512
                for _ in range(warm):
                    nc.tensor.matmul(
                        ps[:, 0:P], ident[:], ident[:], start=True, stop=True
                    )
                for k in range(KC):
                    nc.tensor.matmul(
                        ps[:],
                        w_sb[k][:, ts(m, P)],
                        xt_sb[k][:, ts(nq, 512)],
                        start=(k == 0),
                        stop=(k == KC - 1),
                    )
                    if stepped and k % 2 == 1:
                        yield
                nc.vector.tensor_scalar_add(
                    dst[:, ts(nq, 512)], ps[:], bias[:]
                )
                if stepped:
                    yield

            def vtrans_steps(p, psum_pool, j0=0):
                """PE-transpose KVT chunks into natural-layout V tiles."""
                for j in range(j0, SC):
                    pst = psum_pool.tile(
                        [P, P], BF16, tag="mi", name="vt",
                        padded_shape=[P, 1024],
                    )
                    nc.tensor.transpose(
                        pst[:], kvT_sb[p][:, ts(j, P)], ident[:]
                    )
                    for hl in range(2):
                        nc.vector.tensor_copy(
                            v_sb[p][hl][j][:, 0:HD], pst[:, ts(hl, HD)]
                        )
                    if j % 2 == 1:
                        yield

            # ---- prologue: the minimum before exps can flow: qT m0 cols
            # 0:512 ((0,0) scores rhs) and KVT m0 cols 0:1024 (scores lhsT
            # for j<8 plus the first V transposes).
            with tc.tile_pool(name="psum_pro", bufs=4, space="PSUM") as psum_pro:
                for gen in (
                    proj512(wq_sb, qT_sb[0], bq_sb[0], 0, 0, psum_pro, False, warm=20),
                    proj512(wv_sb, kvT_sb[0], bv_sb[0], 0, 0, psum_pro, False, warm=12),
                ):
                    for _ in gen:
                        pass

            # ---- attention ---------------------------------------------------
            with (
                tc.tile_pool(name="psum_sc", bufs=2, space="PSUM") as psum_sc,
                tc.tile_pool(name="psum_at", bufs=2, space="PSUM") as psum_at,
                tc.tile_pool(name="psum_mi", bufs=2, space="PSUM") as psum_mi,
            ):
                # Preseed the first two V chunks so (0,0)'s first attnT
                # matmuls have emitted writers (Tile deps follow trace order).
                vt0 = vtrans_steps(0, psum_mi, j0=0)
                next(vt0)  # chunks 0,1

                def adv(g, n):
                    for _ in range(n):
                        try:
                            next(g)
                        except StopIteration:
                            return

                def fill00():
                    """(0,0) filler with explicit RAW-safe ordering: KVT m0
                    col-slices must be emitted before the V transposes (and
                    scores) that read them; V chunk j before attnT-j."""
                    kv512 = proj512(wv_sb, kvT_sb[0], bv_sb[0], 0, 512, psum_mi, True)
                    kv1024 = proj512(wv_sb, kvT_sb[0], bv_sb[0], 0, 1024, psum_mi, True)
                    kv1536 = proj512(wv_sb, kvT_sb[0], bv_sb[0], 0, 1536, psum_mi, True)
                    q512 = proj512(wq_sb, qT_sb[0], bq_sb[0], 0, 512, psum_mi, True)
                    adv(kv512, 2); yield
                    adv(kv512, 2); yield
                    adv(kv512, 1); adv(vt0, 1); yield
                    adv(kv1024, 2); yield
                    adv(kv1024, 2); adv(vt0, 1); yield
                    adv(kv1024, 1); adv(vt0, 1); yield
                    adv(kv1536, 2); adv(vt0, 1); yield
                    adv(kv1536, 2); adv(vt0, 1); yield
                    adv(kv1536, 1); adv(vt0, 1); yield
                    adv(vt0, 1); yield
                    adv(q512, 2); yield
                    adv(q512, 2); yield
                    adv(q512, 1); yield

                # Filler schedule: each piece lands in the latest group that
                # still meets its consumer's deadline, so no group is
                # overloaded and the ACT exp stream stays dense.
                fillers = {
                    (0, 0): [fill00()],
                    (0, 1): [
                        proj512(wq_sb, qT_sb[0], bq_sb[0], 0, 1024, psum_mi, True),
                        proj512(wv_sb, kvT_sb[1], bv_sb[1], 1, 0, psum_mi, True),
                        proj512(wv_sb, kvT_sb[1], bv_sb[1], 1, 512, psum_mi, True),
                    ],
                    (0, 2): [
                        proj512(wq_sb, qT_sb[0], bq_sb[0], 0, 1536, psum_mi, True),
                        proj512(wv_sb, kvT_sb[1], bv_sb[1], 1, 1024, psum_mi, True),
                        proj512(wv_sb, kvT_sb[1], bv_sb[1], 1, 1536, psum_mi, True),
                    ],
                    (0, 3): [
                        proj512(wq_sb, qT_sb[1], bq_sb[1], 1, 0, psum_mi, True),
                        vtrans_steps(1, psum_mi),
                    ],
                    (1, 0): [
                        proj512(wq_sb, qT_sb[1], bq_sb[1], 1, 512, psum_mi, True),
                    ],
                    (1, 1): [
                        proj512(wq_sb, qT_sb[1], bq_sb[1], 1, 1024, psum_mi, True),
                    ],
                    (1, 2): [
                        proj512(wq_sb, qT_sb[1], bq_sb[1], 1, 1536, psum_mi, True),
                    ],
                }

                def emit_epilogue(p, q, at):
                    for hl in range(2):
                        head = 2 * p + hl
                        asb = epi_pool.tile([HD, SQ], F32, tag="asb", name="asb")
                        nc.vector.tensor_copy(asb[:], at[hl][0:HD, :])
                        # partition_broadcast reads the tensor's partition 0
                        # regardless of AP offset: stage the denom row in a
                        # dedicated p0 tile first.
                        dr = epi_pool.tile([1, SQ], F32, tag="dr", name="dr")
                        nc.vector.tensor_copy(dr[:], at[hl][HD : HD + 1, :])
                        bc = epi_pool.tile([HD, SQ], F32, tag="bc", name="bc")
                        nc.gpsimd.partition_broadcast(bc[:], dr[:])
                        rc = epi_pool.tile([HD, SQ], F32, tag="rc", name="rc")
                        nc.vector.reciprocal_approx_fast(rc[:], bc[:])
                        ot = epi_pool.tile([HD, SQ], F32, tag="ot", name="ot")
                        nc.vector.tensor_mul(ot[:], asb[:], rc[:])
                        nc.gpsimd.dma_start(out_d[ts(head, HD), ts(q, SQ)], ot[:])

                # Software-pipelined: attnT for slot i is emitted during slot
                # i+1, so the next group's scores/exp never sit behind the
                # previous group's last attnT in PE program order.
                slots = [(p, q, j) for p in range(2) for q in range(NSQ)
                         for j in range(SC)]
                gens = []
                at = None
                prev = None
                for p, q, j in slots:
                    if j == 0:
                        gens = fillers.get((p, q), []) + gens
                        at = [
                            psum_at.tile([HD + 1, SQ], F32, tag="at", name="at")
                            for _ in range(2)
                        ]
                    sc = psum_sc.tile([P, 1024], F32, tag="sc", name="sc")
                    for hl in range(2):
                        nc.tensor.matmul(
                            sc[:, ts(hl, SQ)],
                            kvT_sb[p][hl * HD : (hl + 1) * HD, ts(j, P)],
                            qT_sb[p][hl * HD : (hl + 1) * HD, ts(q, SQ)],
                            start=True,
                            stop=True,
                            tile_position=(hl * HD, 0),
                        )
                    pt = pt_pool.tile([P, 1024], BF16, tag="pt", name="pt")
                    nc.scalar.activation(pt[:], sc[:], Act.Exp, scale=0.125)
                    # filler work (remaining projections, V transposes); one
                    # step per slot keeps PE bursts smaller than the exp time
                    if gens:
                        g = gens.pop(0)
                        try:
                            next(g)
                            gens.append(g)
                        except StopIteration:
                            pass
                    if prev is not None:
                        pp, pq, pj, pat, ppt = prev
                        for hl in range(2):
                            nc.tensor.matmul(
                                pat[hl][:],
                                v_sb[pp][hl][pj][:],
                                ppt[:, ts(hl, SQ)],
                                start=(pj == 0),
                                stop=(pj == SC - 1),
                            )
                        if pj == SC - 1:
                            emit_epilogue(pp, pq, pat)
                    prev = (p, q, j, at, pt)
                # flush the last slot
                p, q, j, at, pt = prev
                for hl in range(2):
                    nc.tensor.matmul(
                        at[hl][:],
                        v_sb[p][hl][j][:],
                        pt[:, ts(hl, SQ)],
                        start=False,
                        stop=True,
                    )
                emit_epilogue(p, q, at)

    nc.compile()
    _NC_CACHE["nc"] = nc
    return nc


def shard_inputs(x, Wq, bq, Wv, bv):
    bf16 = ml_dtypes.bfloat16
    x = np.asarray(x, dtype=np.float32)
    Wq = np.asarray(Wq, dtype=np.float32)
    bq = np.asarray(bq, dtype=np.float32)
    Wv = np.asarray(Wv, dtype=np.float32)
    bv = np.asarray(bv, dtype=np.float32)
    in_maps = []
    xT = [np.ascontiguousarray(x[b].T).astype(bf16) for b in range(B)]
    for c in range(N_CORES):
        b, g = divmod(c, N_CORES // B)
        heads = [HEADS_PER_CORE * g + hl for hl in range(HEADS_PER_CORE)]
        perm = np.array([i * NH + h for h in heads for i in range(HD)])
        in_maps.append(
            {
                "xT": xT[b],
                "wqT": np.ascontiguousarray(Wq[perm, :].T).astype(bf16),
                "wvT": np.ascontiguousarray(Wv[perm, :].T).astype(bf16),
                "bq": np.ascontiguousarray(bq[perm].reshape(DPC, 1)),
                "bv": np.ascontiguousarray(bv[perm].reshape(DPC, 1)),
            }
        )
    return in_maps


def assemble(results):
    out = np.empty((B, S, D), dtype=np.float32)
    for c in range(N_CORES):
        b, g = divmod(c, N_CORES // B)
        out[b][:, g * DPC : (g + 1) * DPC] = results[c]["out"].T
    return out


def kernel(x, Wq, bq, Wv, bv):
    from concourse.bass_utils import run_bass_kernel_spmd

    nc = build_nc()
    in_maps = shard_inputs(x, Wq, bq, Wv, bv)
    res = run_bass_kernel_spmd(nc, in_maps, core_ids=list(range(N_CORES)))
    return assemble(res.results)


if __name__ == "__main__":
    rng = np.random.default_rng(0)
    inputs = {
        "x": rng.standard_normal((B, S, D), dtype=np.float32),
        "Wq": (rng.standard_normal((D, D), dtype=np.float32) / 32.0),
        "bq": rng.standard_normal(D, dtype=np.float32) * 0.02,
        "Wv": (rng.standard_normal((D, D), dtype=np.float32) / 32.0),
        "bv": rng.standard_normal(D, dtype=np.float32) * 0.02,
    }
    out = kernel(**inputs)
    print("kernel ran, out shape:", out.shape)


# revision 22
# speedup vs baseline: 1.7847x; 1.0105x over previous
"""Trainium2 Bass kernel for nn_Attention_70136815943694.

Attention with the reference's source bug preserved (K uses the V
projection). x:[2,2048,1024], 16 heads x 64 dim. Sharded over 8
NeuronCores as (batch x head-group): core c handles batch c//4 and
heads [4*(c%4) .. 4*(c%4)+3]. Each core's output slice is independent,
so there are no collectives; the host shards inputs and reassembles.

Per-core device pipeline (d-major layouts):
  QT = wqT.T @ xT (+bq)     [256, 2048] bf16   (DVE evac w/ bias)
  KVT = wvT.T @ xT (+bv)    [256, 2048] bf16
  V   = PE-transpose of KVT chunks (bias already included)
  per head-pair p, s1 quarter q (512 wide), s2 chunk j (128):
    scores: two K=64 matmuls row-packed via tile_position (0,0)/(64,0)
    PT = exp(0.125*scores) on ACT, one [128,1024] op for both heads
    atH += [V_h | 1].T @ PT_h   [65, 512] psum, row 64 = softmax denom
  epilogue: gpsimd partition_broadcast denom, DVE fast reciprocal, mul
The attention phase is ACT(exp)-bound; all projection/V work beyond the
minimal prologue (first halves of QT/KVT m=0) is interleaved into the
attention groups as PE filler so the exp stream starts ~as soon as the
input DMAs land and never starves.
"""
import numpy as np
import ml_dtypes

B = 2
S = 2048
D = 1024
NH = 16
HD = 64
N_CORES = 8
HEADS_PER_CORE = 4
DPC = HEADS_PER_CORE * HD  # 256 projection rows per core
P = 128
KC = D // P  # 8 contraction chunks
SC = S // P  # 16 s2 chunks
SQ = 512  # s1 quarter width
NSQ = S // SQ  # 4

_NC_CACHE = {}


def build_nc():
    if "nc" in _NC_CACHE:
        return _NC_CACHE["nc"]
    import concourse.bass as bass
    import concourse.mybir as mybir
    import concourse.tile as tile
    from concourse import bacc
    from concourse.masks import make_identity

    BF16 = mybir.dt.bfloat16
    F32 = mybir.dt.float32
    Act = mybir.ActivationFunctionType
    ts = bass.ts

    nc = bacc.Bacc(None, target_bir_lowering=False, debug=False)
    xT_d = nc.declare_dram_parameter("xT", [D, S], BF16, isOutput=False)
    wqT_d = nc.declare_dram_parameter("wqT", [D, DPC], BF16, isOutput=False)
    wvT_d = nc.declare_dram_parameter("wvT", [D, DPC], BF16, isOutput=False)
    bq_d = nc.declare_dram_parameter("bq", [DPC, 1], F32, isOutput=False)
    bv_d = nc.declare_dram_parameter("bv", [DPC, 1], F32, isOutput=False)
    out_d = nc.declare_dram_parameter("out", [DPC, S], F32, isOutput=True)

    with tile.TileContext(nc) as tc:
        with (
            tc.tile_pool(name="persist", bufs=1) as persist,
            tc.tile_pool(name="pt", bufs=8) as pt_pool,
            tc.tile_pool(name="epi", bufs=2) as epi_pool,
        ):
            # warm the ACT exp table set at t~0 so the one-time table load
            # overlaps the input DMAs
            warm = persist.tile([1, 8], F32, tag="warm")
            nc.vector.memset(warm[:], 0.0)
            nc.scalar.activation(warm[:], warm[:], Act.Exp, scale=1.0)

            ident = persist.tile([P, P], BF16, tag="ident")
            make_identity(nc, ident[:])

            # ---- input loads: xT on the sync HWDGE queue, weights/biases on
            # the gpsimd SWDGE queue so the issue streams run in parallel.
            xt_sb = [
                persist.tile([P, S], BF16, name=f"xt{k}", tag=f"xt{k}")
                for k in range(KC)
            ]
            wq_sb = [
                persist.tile([P, DPC], BF16, name=f"wq{k}", tag=f"wq{k}")
                for k in range(KC)
            ]
            wv_sb = [
                persist.tile([P, DPC], BF16, name=f"wv{k}", tag=f"wv{k}")
                for k in range(KC)
            ]
            bq_sb = [
                persist.tile([P, 1], F32, name=f"bq{m}", tag=f"bq{m}")
                for m in range(2)
            ]
            bv_sb = [
                persist.tile([P, 1], F32, name=f"bv{m}", tag=f"bv{m}")
                for m in range(2)
            ]
            # wq0/wv0 first (small, unblock the first LDWEIGHTS), then xT in
            # column-quarters, column-major: the prologue only reads cols
            # 0:512 of every k-chunk, so its 1MB lands in ~3us instead of
            # waiting for the full 4MB.
            nc.sync.dma_start(wq_sb[0][:], wqT_d[ts(0, P), :])
            nc.sync.dma_start(wv_sb[0][:], wvT_d[ts(0, P), :])
            for cq in range(2):
                for k in range(KC):
                    nc.sync.dma_start(
                        xt_sb[k][:, ts(cq, 512)], xT_d[ts(k, P), ts(cq, 512)]
                    )
            for k in range(KC):
                nc.sync.dma_start(
                    xt_sb[k][:, 1024:2048], xT_d[ts(k, P), 1024:2048]
                )
            for k in range(1, KC):
                nc.gpsimd.dma_start(wq_sb[k][:], wqT_d[ts(k, P), :])
                nc.gpsimd.dma_start(wv_sb[k][:], wvT_d[ts(k, P), :])
            for m in range(2):
                nc.gpsimd.dma_start(bq_sb[m][:], bq_d[ts(m, P), :])
                nc.gpsimd.dma_start(bv_sb[m][:], bv_d[ts(m, P), :])

            qT_sb = [
                persist.tile([P, S], BF16, name=f"qT{m}", tag=f"qT{m}")
                for m in range(2)
            ]
            kvT_sb = [
                persist.tile([P, S], BF16, name=f"kvT{m}", tag=f"kvT{m}")
                for m in range(2)
            ]
            # v_sb[p][hl][j]: [128, 65] = V chunk j for head 2p+hl, col 64 = 1
            v_sb = [
                [
                    [
                        persist.tile(
                            [P, HD + 1], BF16,
                            name=f"v{p}_{hl}_{j}", tag=f"v{p}_{hl}_{j}",
                        )
                        for j in range(SC)
                    ]
                    for hl in range(2)
                ]
                for p in range(2)
            ]
            for p in range(2):
                for hl in range(2):
                    for j in range(SC):
                        nc.vector.memset(v_sb[p][hl][j][:, HD : HD + 1], 1.0)

            def proj512(w_sb, dst, bias, m, c0, psum_pool, stepped, warm=0):
                """One 512-col slice [c0:c0+512] of a projection m-chunk.
                warm>0 emits that many throwaway ident matmuls into the psum
                first (overwritten by the real k0 start=True) to lift the PE
                HAM clock gate during the DMA-bound start."""
                ps = psum_pool.tile([P, 512], F32, tag="mi", name="pp")
                nq = c0 // 512
                for _ in range(warm):
                    nc.tensor.matmul(
                        ps[:, 0:P], ident[:], ident[:], start=True, stop=True
                    )
                for k in range(KC):
                    nc.tensor.matmul(
                        ps[:],
                        w_sb[k][:, ts(m, P)],
                        xt_sb[k][:, ts(nq, 512)],
                        start=(k == 0),
                        stop=(k == KC - 1),
                    )
                    if stepped and k % 2 == 1:
                        yield
                nc.vector.tensor_scalar_add(
                    dst[:, ts(nq, 512)], ps[:], bias[:]
                )
                if stepped:
                    yield

            def vtrans_steps(p, psum_pool, j0=0):
                """PE-transpose KVT chunks into natural-layout V tiles."""
                for j in range(j0, SC):
                    pst = psum_pool.tile(
                        [P, P], BF16, tag="mi", name="vt",
                        padded_shape=[P, 1024],
                    )
                    nc.tensor.transpose(
                        pst[:], kvT_sb[p][:, ts(j, P)], ident[:]
                    )
                    for hl in range(2):
                        nc.vector.tensor_copy(
                            v_sb[p][hl][j][:, 0:HD], pst[:, ts(hl, HD)]
                        )
                    if j % 2 == 1:
                        yield

            # ---- prologue: the minimum before exps can flow: qT m0 cols
            # 0:512 ((0,0) scores rhs) and KVT m0 cols 0:1024 (scores lhsT
            # for j<8 plus the first V transposes).
            with tc.tile_pool(name="psum_pro", bufs=4, space="PSUM") as psum_pro:
                for gen in (
                    proj512(wq_sb, qT_sb[0], bq_sb[0], 0, 0, psum_pro, False, warm=10),
                    proj512(wv_sb, kvT_sb[0], bv_sb[0], 0, 0, psum_pro, False),
                ):
                    for _ in gen:
                        pass

            # ---- attention ---------------------------------------------------
            with (
                tc.tile_pool(name="psum_sc", bufs=2, space="PSUM") as psum_sc,
                tc.tile_pool(name="psum_at", bufs=2, space="PSUM") as psum_at,
                tc.tile_pool(name="psum_mi", bufs=2, space="PSUM") as psum_mi,
            ):
                # Preseed the first two V chunks so (0,0)'s first attnT
                # matmuls have emitted writers (Tile deps follow trace order).
                vt0 = vtrans_steps(0, psum_mi, j0=0)
                next(vt0)  # chunks 0,1

                def adv(g, n):
                    for _ in range(n):
                        try:
                            next(g)
                        except StopIteration:
                            return

                def fill00():
                    """(0,0) filler with explicit RAW-safe ordering: KVT m0
                    col-slices must be emitted before the V transposes (and
                    scores) that read them; V chunk j before attnT-j."""
                    kv512 = proj512(wv_sb, kvT_sb[0], bv_sb[0], 0, 512, psum_mi, True)
                    kv1024 = proj512(wv_sb, kvT_sb[0], bv_sb[0], 0, 1024, psum_mi, True)
                    kv1536 = proj512(wv_sb, kvT_sb[0], bv_sb[0], 0, 1536, psum_mi, True)
                    q512 = proj512(wq_sb, qT_sb[0], bq_sb[0], 0, 512, psum_mi, True)
                    adv(kv512, 2); yield
                    adv(kv512, 2); yield
                    adv(kv512, 1); adv(vt0, 1); yield
                    adv(kv1024, 2); yield
                    adv(kv1024, 2); adv(vt0, 1); yield
                    adv(kv1024, 1); adv(vt0, 1); yield
                    adv(kv1536, 2); adv(vt0, 1); yield
                    adv(kv1536, 2); adv(vt0, 1); yield
                    adv(kv1536, 1); adv(vt0, 1); yield
                    adv(vt0, 1); yield
                    adv(q512, 2); yield
                    adv(q512, 2); yield
                    adv(q512, 1); yield

                # Filler schedule: each piece lands in the latest group that
                # still meets its consumer's deadline, so no group is
                # overloaded and the ACT exp stream stays dense.
                fillers = {
                    (0, 0): [fill00()],
                    (0, 1): [
                        proj512(wq_sb, qT_sb[0], bq_sb[0], 0, 1024, psum_mi, True),
                        proj512(wv_sb, kvT_sb[1], bv_sb[1], 1, 0, psum_mi, True),
                        proj512(wv_sb, kvT_sb[1], bv_sb[1], 1, 512, psum_mi, True),
                    ],
                    (0, 2): [
                        proj512(wq_sb, qT_sb[0], bq_sb[0], 0, 1536, psum_mi, True),
                        proj512(wv_sb, kvT_sb[1], bv_sb[1], 1, 1024, psum_mi, True),
                        proj512(wv_sb, kvT_sb[1], bv_sb[1], 1, 1536, psum_mi, True),
                    ],
                    (0, 3): [
                        proj512(wq_sb, qT_sb[1], bq_sb[1], 1, 0, psum_mi, True),
                        vtrans_steps(1, psum_mi),
                    ],
                    (1, 0): [
                        proj512(wq_sb, qT_sb[1], bq_sb[1], 1, 512, psum_mi, True),
                    ],
                    (1, 1): [
                        proj512(wq_sb, qT_sb[1], bq_sb[1], 1, 1024, psum_mi, True),
                    ],
                    (1, 2): [
                        proj512(wq_sb, qT_sb[1], bq_sb[1], 1, 1536, psum_mi, True),
                    ],
                }

                def emit_epilogue(p, q, at):
                    for hl in range(2):
                        head = 2 * p + hl
                        asb = epi_pool.tile([HD, SQ], F32, tag="asb", name="asb")
                        nc.vector.tensor_copy(asb[:], at[hl][0:HD, :])
                        # partition_broadcast reads the tensor's partition 0
                        # regardless of AP offset: stage the denom row in a
                        # dedicated p0 tile first.
                        dr = epi_pool.tile([1, SQ], F32, tag="dr", name="dr")
                        nc.vector.tensor_copy(dr[:], at[hl][HD : HD + 1, :])
                        bc = epi_pool.tile([HD, SQ], F32, tag="bc", name="bc")
                        nc.gpsimd.partition_broadcast(bc[:], dr[:])
                        rc = epi_pool.tile([HD, SQ], F32, tag="rc", name="rc")
                        nc.vector.reciprocal_approx_fast(rc[:], bc[:])
                        ot = epi_pool.tile([HD, SQ], F32, tag="ot", name="ot")
                        nc.vector.tensor_mul(ot[:], asb[:], rc[:])
                        nc.sync.dma_start(out_d[ts(head, HD), ts(q, SQ)], ot[:])

                # Software-pipelined: attnT for slot i is emitted during slot
                # i+1, so the next group's scores/exp never sit behind the
                # previous group's last attnT in PE program order.
                slots = [(p, q, j) for p in range(2) for q in range(NSQ)
                         for j in range(SC)]
                gens = []
                at = None
                prev = None
                for p, q, j in slots:
                    if j == 0:
                        gens = fillers.get((p, q), []) + gens
                        at = [
                            psum_at.tile([HD + 1, SQ], F32, tag="at", name="at")
                            for _ in range(2)
                        ]
                    sc = psum_sc.tile([P, 1024], F32, tag="sc", name="sc")
                    for hl in range(2):
                        nc.tensor.matmul(
                            sc[:, ts(hl, SQ)],
                            kvT_sb[p][hl * HD : (hl + 1) * HD, ts(j, P)],
                            qT_sb[p][hl * HD : (hl + 1) * HD, ts(q, SQ)],
                            start=True,
                            stop=True,
                            tile_position=(hl * HD, 0),
                        )
                    pt = pt_pool.tile([P, 1024], BF16, tag="pt", name="pt")
                    nc.scalar.activation(pt[:], sc[:], Act.Exp, scale=0.125)
                    # filler work (remaining projections, V transposes); one
                    # step per slot keeps PE bursts smaller than the exp time
                    if gens:
                        g = gens.pop(0)
                        try:
                            next(g)
                            gens.append(g)
                        except StopIteration:
                            pass
                    if prev is not None:
                        pp, pq, pj, pat, ppt = prev
                        for hl in range(2):
                            nc.tensor.matmul(
                                pat[hl][:],
                                v_sb[pp][hl][pj][:],
                                ppt[:, ts(hl, SQ)],
                                start=(pj == 0),
                                stop=(pj == SC - 1),
                            )
                        if pj == SC - 1:
                            emit_epilogue(pp, pq, pat)
                    prev = (p, q, j, at, pt)
                # flush the last slot
                p, q, j, at, pt = prev
                for hl in range(2):
                    nc.tensor.matmul(
                        at[hl][:],
                        v_sb[p][hl][j][:],
                        pt[:, ts(hl, SQ)],
                        start=False,
                        stop=True,
                    )
                emit_epilogue(p, q, at)

    nc.compile()
    _NC_CACHE["nc"] = nc
    return nc


def shard_inputs(x, Wq, bq, Wv, bv):
    bf16 = ml_dtypes.bfloat16
    x = np.asarray(x, dtype=np.float32)
    Wq = np.asarray(Wq, dtype=np.float32)
    bq = np.asarray(bq, dtype=np.float32)
    Wv = np.asarray(Wv, dtype=np.float32)
    bv = np.asarray(bv, dtype=np.float32)
    in_maps = []
    xT = [np.ascontiguousarray(x[b].T).astype(bf16) for b in range(B)]
    for c in range(N_CORES):
        b, g = divmod(c, N_CORES // B)
        heads = [HEADS_PER_CORE * g + hl for hl in range(HEADS_PER_CORE)]
        perm = np.array([i * NH + h for h in heads for i in range(HD)])
        in_maps.append(
            {
                "xT": xT[b],
                "wqT": np.ascontiguousarray(Wq[perm, :].T).astype(bf16),
                "wvT": np.ascontiguousarray(Wv[perm, :].T).astype(bf16),
                "bq": np.ascontiguousarray(bq[perm].reshape(DPC, 1)),
                "bv": np.ascontiguousarray(bv[perm].reshape(DPC, 1)),
            }
        )
    return in_maps


def assemble(results):
    out = np.empty((B, S, D), dtype=np.float32)
    for c in range(N_CORES):
        b, g = divmod(c, N_CORES // B)
        out[b][:, g * DPC : (g + 1) * DPC] = results[c]["out"].T
    return out


def kernel(x, Wq, bq, Wv, bv):
    from concourse.bass_utils import run_bass_kernel_spmd

    nc = build_nc()
    in_maps = shard_inputs(x, Wq, bq, Wv, bv)
    res = run_bass_kernel_spmd(nc, in_maps, core_ids=list(range(N_CORES)))
    return assemble(res.results)


if __name__ == "__main__":
    rng = np.random.default_rng(0)
    inputs = {
        "x": rng.standard_normal((B, S, D), dtype=np.float32),
        "Wq": (rng.standard_normal((D, D), dtype=np.float32) / 32.0),
        "bq": rng.standard_normal(D, dtype=np.float32) * 0.02,
        "Wv": (rng.standard_normal((D, D), dtype=np.float32) / 32.0),
        "bv": rng.standard_normal(D, dtype=np.float32) * 0.02,
    }
    out = kernel(**inputs)
    print("kernel ran, out shape:", out.shape)
